# revision 28
# baseline (speedup 1.0000x reference)
"""GQA attention prefill kernel for Trainium2 (Bass/Tile), 8-way tensor
parallel over heads.

Problem (hardcoded): B=1, S=2048, HID=4096, NH=32, KVH=8, D=128, causal
prefill with per-head RMSNorm on q/k and RoPE, positions = arange(S).

Sharding: core c owns kv-head c and q-heads 4c..4c+3. wq/wo sharded on the
head dim, wk/wv on the kv-head dim; x, rope tables replicated. Each core
computes its 4 heads' contribution through wo; the host sums the 8 partial
outputs (partials shipped as fp16, summed in fp32 on host).

Host-side prep (part of sharding): all matmul operands are fed pre-
transposed (contraction dim major) and pre-converted to fp16 — fp16
matmuls run at the same 1 cycle/row as fp32r but halve DMA and SBUF. The
q/k norm weights, the rotate-half sign, and the softmax 1/sqrt(d) are
folded into the rope tables; the sin tables are additionally pre-rotated
by 64 so the on-device rotate-half becomes a single PE matmul against a
fixed permutation matrix (no SBUF->SBUF DMAs).

Two on-device phases to keep PSUM-bank pressure per phase under the 8
banks (cross-phase overlap then never deadlocks the scheduler):
  1. per 512-column chunk: stream x once (quarter-chunk DMAs), project
     q/k/v (v directly transposed via x-as-stationary matmuls), rope +
     rmsnorm into resident KR / qr tiles. rsqrt is computed as
     exp(-0.5*ln(.)): with a manually preloaded combined {ln,exp,copy,
     square} activation table set the whole kernel needs exactly one
     ACT table load.
  2. causal attention per chunk (exp'd score tiles in bf16 — fp16 would
     overflow at score ~ +11.4; softmax denominators accumulate on the
     otherwise-idle Pool engine, one f32 ones-matmul per (head, chunk)
     folds the partitions) followed by the wo matmul; y partials are
     staged in fp16 and written by DMAs issued from the Pool queue so
     the SP queue never delays x prefetches.
"""

import numpy as np

import concourse.bass as bass
import concourse.mybir as mybir
import concourse.tile as tile
from concourse import bacc

P = 128
S = 2048
HID = 4096
D = 128
G = 4            # q heads per core
NHT = HID // P   # 32 h-tiles (contraction)
SC = 512         # seq chunk
NSC = S // SC    # 4
NKT = S // P     # 16 k-tiles
EPS = 1e-6
N_CORES = 8
WQKV = G * P + 2 * P   # 768 packed projection output dims (q|k|v)

F32 = mybir.dt.float32
F16 = mybir.dt.float16
BF16 = mybir.dt.bfloat16

ACT_SET_LN_EXP = 6     # natural_log_exp_and_others: {ln, exp, copy, square}


def build_program(dbg=False):
    nc = bacc.Bacc("TRN2", target_bir_lowering=False, debug=False)
    if dbg:
        KR_dbg = nc.dram_tensor("KR_dbg", [P, S], F16,
                                kind="ExternalOutput").ap()
        QR_dbg = nc.dram_tensor("QR_dbg", [P, G, S], F16,
                                kind="ExternalOutput").ap()
        V_dbg = nc.dram_tensor("V_dbg", [P, NKT, P], BF16,
                               kind="ExternalOutput").ap()
        OT_dbg = nc.dram_tensor("OT_dbg", [P, G, S], F16,
                                kind="ExternalOutput").ap()

    xT = nc.dram_tensor("xT", [HID, S], F16, kind="ExternalInput").ap()
    wqkvT = nc.dram_tensor("wqkvT", [HID, WQKV], F16, kind="ExternalInput").ap()
    woT = nc.dram_tensor("woT", [G * P, HID], F16, kind="ExternalInput").ap()
    cosq = nc.dram_tensor("cosq", [D, S], F16, kind="ExternalInput").ap()
    sinq = nc.dram_tensor("sinq", [D, S], F16, kind="ExternalInput").ap()
    cosk = nc.dram_tensor("cosk", [D, S], F16, kind="ExternalInput").ap()
    sink = nc.dram_tensor("sink", [D, S], F16, kind="ExternalInput").ap()
    y = nc.dram_tensor("y", [S, HID], F16, kind="ExternalOutput").ap()

    Ln = mybir.ActivationFunctionType.Ln
    Exp = mybir.ActivationFunctionType.Exp

    with tile.TileContext(nc) as tc:
        with (
            tc.tile_pool(name="const", bufs=1) as const,
            tc.tile_pool(name="tabs", bufs=2) as tabs,
            tc.tile_pool(name="xw", bufs=5) as xw,
            tc.tile_pool(name="scr", bufs=2) as scr,
            tc.tile_pool(name="rawp", bufs=6) as rawp,
            tc.tile_pool(name="otp", bufs=5) as otp,
            tc.tile_pool(name="ptp", bufs=4) as ptp,
            tc.tile_pool(name="dnp", bufs=4) as dnp,
            tc.tile_pool(name="yp", bufs=3) as yp,
            tc.tile_pool(name="ps", bufs=6, space="PSUM") as ps,
            tc.tile_pool(name="psd", bufs=2, space="PSUM") as psd,
        ):
            # one activation-table load for the whole kernel
            nc.scalar.add_instruction(mybir.InstLoadActFuncSet(
                name=nc.get_next_instruction_name(), ins=[], outs=[],
                act_func_set_id=ACT_SET_LN_EXP))

            # ---- input DMAs, interleaved so x quarters and the weight
            # parts feeding the first projections arrive together; the
            # later weight parts and woT go on the idle ACT queue ----
            xTr = xT.rearrange("(a p) s -> p a s", p=P)
            wqkv_sb = const.tile([P, NHT, WQKV], F16)
            wqkvTr = wqkvT.rearrange("(a p) c -> p a c", p=P)
            xs_first = []
            for qi in range(4):
                xs = xw.tile([P, 8, SC], F16, tag="xs")
                nc.sync.dma_start(xs, xTr[:, 8 * qi:8 * (qi + 1), 0:SC])
                xs_first.append(xs)
                nc.sync.dma_start(wqkv_sb[:, 8 * qi:8 * qi + 4, :],
                                  wqkvTr[:, 8 * qi:8 * qi + 4, :])
                nc.scalar.dma_start(wqkv_sb[:, 8 * qi + 4:8 * qi + 8, :],
                                    wqkvTr[:, 8 * qi + 4:8 * qi + 8, :])
            woT_sb = const.tile([P, G, HID], F16)
            nc.scalar.dma_start(
                woT_sb, woT.rearrange("(g p) h -> p g h", p=P))

            # ---- constants ----------------------------------------------
            f32tmp = const.tile([P, SC], F32)
            f32tmp2 = const.tile([P, P], F32)

            # ones[k, m] == 1: matmul(out, ones, rhs) -> column sums of rhs
            # broadcast across all 128 output partitions.
            ones16 = const.tile([P, P], F16)
            nc.gpsimd.memset(f32tmp, 1.0)
            nc.vector.tensor_copy(ones16, f32tmp[:, 0:P])
            ones32 = const.tile([P, P], F32)
            nc.gpsimd.memset(ones32, 1.0)

            # rotP: lhsT permutation with rotP[k, i] = 1 iff i == (k+64)%128,
            # so matmul(out, rotP, v) = v rotated-half along partitions.
            nc.gpsimd.memset(f32tmp2, 1.0)
            rsel1 = const.tile([P, P], F32)
            nc.gpsimd.affine_select(
                rsel1, f32tmp2, pattern=[[1, P]],
                compare_op=mybir.AluOpType.is_equal,
                fill=0.0, base=-64, channel_multiplier=-1,
            )
            rsel2 = const.tile([P, P], F32)
            nc.gpsimd.affine_select(
                rsel2, f32tmp2, pattern=[[1, P]],
                compare_op=mybir.AluOpType.is_equal,
                fill=0.0, base=64, channel_multiplier=-1,
            )
            nc.vector.tensor_add(rsel1, rsel1, rsel2)
            rotP = const.tile([P, P], F16)
            nc.vector.tensor_copy(rotP, rsel1)

            # causal masks for the 4 diagonal k-tiles of a q chunk:
            # keep (1.0) where q_local >= 128*j + k_local
            masks = []
            for j in range(4):
                mk = const.tile([P, SC], BF16, name=f"mask{j}")
                nc.gpsimd.memset(f32tmp, 1.0)
                nc.gpsimd.affine_select(
                    f32tmp, f32tmp, pattern=[[1, SC]],
                    compare_op=mybir.AluOpType.is_ge,
                    fill=0.0, base=-P * j, channel_multiplier=-1,
                )
                nc.vector.tensor_copy(mk, f32tmp)
                masks.append(mk)

            bias_keps = const.tile([P, 1], F32)
            nc.gpsimd.memset(bias_keps, float(P) * EPS)
            bias_qeps = const.tile([P, 1], F32)
            nc.gpsimd.memset(bias_qeps, EPS)

            # ---- resident tensors ---------------------------------------
            KR = const.tile([P, S], F16)          # roped+scaled K, [d, s]
            Vs = const.tile([P, NKT, P], BF16)    # V, [s-in-tile, k-tile, d]
            qr_all = const.tile([P, G, S], F16)   # roped+scaled Q, [d, h, s]

            # ============ phase 1: projections + rope ====================
            for sc in range(NSC):
                q0 = sc * SC

                # projections: accumulate over 32 h-tiles, x streamed in
                # 4 quarter-chunk tiles of 8 h-tiles each
                qps = [ps.tile([P, SC], F32, tag="ps", name=f"qps{i}")
                       for i in range(G)]
                kps = ps.tile([P, SC], F32, tag="ps")
                xss = []
                for qi in range(4):
                    if sc == 0:
                        xs = xs_first[qi]
                    else:
                        xs = xw.tile([P, 8, SC], F16, tag="xs")
                        nc.sync.dma_start(
                            xs, xTr[:, 8 * qi:8 * (qi + 1), q0:q0 + SC])
                    xss.append(xs)
                    for ht8 in range(8):
                        ht = qi * 8 + ht8
                        xt = xs[:, ht8, :]
                        st = ht == 0
                        sp = ht == NHT - 1
                        for mt in range(G):
                            nc.tensor.matmul(
                                qps[mt],
                                wqkv_sb[:, ht, mt * P:(mt + 1) * P], xt,
                                start=st, stop=sp,
                            )
                        nc.tensor.matmul(
                            kps, wqkv_sb[:, ht, G * P:G * P + P], xt,
                            start=st, stop=sp,
                        )

                # v, directly transposed: out[s, d]; each 128-row s-subtile
                # accumulates in its own small PSUM tile (separate banks -
                # a PSUM bank region only supports one accumulation group
                # at a time)
                for j in range(4):
                    vpsd = psd.tile([P, P], F32, tag="psq")
                    for ht in range(NHT):
                        nc.tensor.matmul(
                            vpsd,
                            xss[ht // 8][:, ht % 8, j * P:(j + 1) * P],
                            wqkv_sb[:, ht, G * P + P:],
                            start=(ht == 0), stop=(ht == NHT - 1),
                        )
                    nc.scalar.copy(Vs[:, sc * 4 + j, :], vpsd)

                cq = tabs.tile([P, SC], F16, tag="cosq")
                nc.sync.dma_start(cq, cosq[:, q0:q0 + SC])
                sq_ = tabs.tile([P, SC], F16, tag="sinq")
                nc.sync.dma_start(sq_, sinq[:, q0:q0 + SC])
                ck = tabs.tile([P, SC], F16, tag="cosk")
                nc.sync.dma_start(ck, cosk[:, q0:q0 + SC])
                sk = tabs.tile([P, SC], F16, tag="sink")
                nc.sync.dma_start(sk, sink[:, q0:q0 + SC])

                # copy all projection outputs out of PSUM first, so the
                # 6 projection banks free up for the next chunk ASAP
                kraw = rawp.tile([P, SC], F16, tag="raw")
                nc.scalar.copy(kraw, kps)
                qraws = []
                for h in range(G):
                    qraw = rawp.tile([P, SC], F16, tag="raw")
                    nc.scalar.copy(qraw, qps[h])
                    qraws.append(qraw)

                # K: rope + fold per-k norm scale into KR columns
                sqk = scr.tile([P, SC], F16, tag="sq")
                nc.vector.tensor_mul(sqk, kraw, kraw)
                ssb = ps.tile([P, SC], F32, tag="ps")
                nc.tensor.matmul(ssb, ones16, sqk, start=True, stop=True)
                # 1/sqrt(ssq + d*eps) == exp(-0.5*ln(ssq + d*eps)):
                # k-norm and softmax 1/sqrt(d) in one factor
                lnk = scr.tile([P, SC], F32, tag="lnk")
                nc.scalar.activation(lnk, ssb, Ln, bias=bias_keps, scale=1.0)
                rkf = scr.tile([P, SC], F32, tag="rk")
                nc.scalar.activation(rkf, lnk, Exp, bias=0.0, scale=-0.5)
                # rope: out = z*cos + rot(z*sin_pre), sin pre-rotated on host
                t1 = scr.tile([P, SC], F16, tag="t1")
                nc.vector.tensor_mul(t1, kraw, sk)
                rps = ps.tile([P, SC], F32, tag="ps")
                nc.tensor.matmul(rps, rotP, t1, start=True, stop=True)
                kpre = scr.tile([P, SC], F32, tag="kpre")
                nc.vector.tensor_mul(kpre, kraw, ck)
                nc.vector.tensor_add(kpre, kpre, rps)
                nc.vector.tensor_mul(KR[:, q0:q0 + SC], kpre, rkf)

                # Q per head: rope + norm scale
                for h in range(G):
                    qraw = qraws[h]
                    sqq = scr.tile([P, SC], F16, tag="sq")
                    nc.vector.tensor_mul(sqq, qraw, qraw)
                    ssbq = ps.tile([P, SC], F32, tag="ps")
                    nc.tensor.matmul(ssbq, ones16, sqq,
                                     start=True, stop=True)
                    lnq = scr.tile([P, SC], F32, tag="lnk")
                    nc.scalar.activation(lnq, ssbq, Ln,
                                         bias=bias_qeps, scale=1.0 / P)
                    rqf = scr.tile([P, SC], F32, tag="rk")
                    nc.scalar.activation(rqf, lnq, Exp, bias=0.0, scale=-0.5)
                    t1b = scr.tile([P, SC], F16, tag="t1")
                    nc.vector.tensor_mul(t1b, qraw, sq_)
                    rpq = ps.tile([P, SC], F32, tag="ps")
                    nc.tensor.matmul(rpq, rotP, t1b, start=True, stop=True)
                    qpre = scr.tile([P, SC], F32, tag="kpre")
                    nc.vector.tensor_mul(qpre, qraw, cq)
                    nc.vector.tensor_add(qpre, qpre, rpq)
                    nc.vector.tensor_mul(qr_all[:, h, q0:q0 + SC],
                                         qpre, rqf)

            # ============ phase 2: attention + output projection =========
            for sc in range(NSC):
                q0 = sc * SC
                ots = []
                off = sc * 4          # full (off-diagonal) k-tiles
                nkt = off + 4
                for h in range(G):
                    qrh = qr_all[:, h, q0:q0 + SC]
                    avp = ps.tile([P, SC], F32, tag="ps")
                    dnacc = dnp.tile([P, SC], F32, tag="dn")
                    for kt in range(nkt):
                        ptps = ps.tile([P, SC], F32, tag="ps")
                        nc.tensor.matmul(
                            ptps, KR[:, kt * P:(kt + 1) * P], qrh,
                            start=True, stop=True,
                        )
                        pt = ptp.tile([P, SC], BF16, tag="pt")
                        nc.scalar.activation(pt, ptps, Exp, bias=0.0,
                                             scale=1.0)
                        if kt >= off:
                            nc.vector.tensor_mul(pt, pt, masks[kt - off])
                        # denominator partial sums on the Pool engine
                        if kt == 0:
                            nc.gpsimd.tensor_copy(dnacc, pt)
                        else:
                            nc.gpsimd.tensor_add(dnacc, dnacc, pt)
                        nc.tensor.matmul(avp, Vs[:, kt, :], pt,
                                         start=(kt == 0), stop=(kt == nkt - 1))
                    # fold the 128 partition-partials of the denominator;
                    # 1/16 rescale (fp16 headroom) is folded into woT on host
                    dn16 = scr.tile([P, SC], F16, tag="dn16")
                    nc.scalar.activation(dn16, dnacc,
                                         mybir.ActivationFunctionType.Copy,
                                         bias=0.0, scale=1.0 / 16.0)
                    dnps = ps.tile([P, SC], F32, tag="ps")
                    nc.tensor.matmul(dnps, ones16, dn16,
                                     start=True, stop=True)
                    rcp = scr.tile([P, SC], F32, tag="lnk")
                    nc.vector.reciprocal(rcp, dnps)
                    ot = otp.tile([P, SC], F16, tag="ot")
                    nc.vector.tensor_mul(ot, avp, rcp)
                    ots.append(ot)
                    if dbg:
                        nc.gpsimd.dma_start(OT_dbg[:, h, q0:q0 + SC], ot)

                # output projection: 4 column groups of 2 PSUM banks
                for stl in range(SC // P):
                    srow = q0 + stl * P
                    for grp in range(4):
                        ybuf = yp.tile([P, 2 * SC], F16, tag="ys")
                        yps_l = [ps.tile([P, SC], F32, tag="ps",
                                         name=f"yps{j}")
                                 for j in range(2)]
                        for h in range(G):
                            lhs = ots[h][:, stl * P:(stl + 1) * P]
                            for j in range(2):
                                hc = grp * 2 + j
                                nc.tensor.matmul(
                                    yps_l[j], lhs,
                                    woT_sb[:, h, hc * SC:(hc + 1) * SC],
                                    start=(h == 0), stop=(h == G - 1),
                                )
                        nc.scalar.copy(ybuf[:, 0:SC], yps_l[0])
                        nc.vector.tensor_copy(ybuf[:, SC:2 * SC], yps_l[1])
                        nc.gpsimd.dma_start(
                            y[srow:srow + P,
                              grp * 2 * SC:(grp + 1) * 2 * SC], ybuf)

            if dbg:
                nc.sync.dma_start(KR_dbg, KR)
                nc.sync.dma_start(QR_dbg, qr_all)
                nc.sync.dma_start(V_dbg, Vs)

    nc.finalize()
    return nc


def shard_inputs(x, wq, wk, wv, wo, q_norm_w, k_norm_w, cos_table, sin_table,
                 positions, **_ignored):
    """Host-side sharding: returns the list of 8 per-core input maps."""
    x = np.asarray(x, np.float32)
    pos = np.asarray(positions).astype(np.int64)
    cos_sel = np.asarray(cos_table, np.float32)[pos]   # [S, D]
    sin_sel = np.asarray(sin_table, np.float32)[pos]
    qw = np.asarray(q_norm_w, np.float32)
    kw = np.asarray(k_norm_w, np.float32)
    # fold norm weights into the transposed rope tables:
    # w * rope(q') == q'*(w*cos) + rot(q')*(w*sin)
    # fold rotate-half's minus sign into sin rows 0..63:
    # rope(z) = z*cos + [-z2; z1]*sin = z*cos + rot(z)*sin_eff
    # and pre-rotate sin so rot(z)*sin_eff == rot(z * rot(sin_eff))
    sign = np.ones((1, D), np.float32)
    sign[0, :D // 2] = -1.0
    cosq_ = np.ascontiguousarray((cos_sel * qw).T).astype(np.float16)
    sinq_ = np.ascontiguousarray(
        np.roll((sin_sel * qw * sign).T, 64, axis=0)).astype(np.float16)
    cosk_ = np.ascontiguousarray((cos_sel * kw).T).astype(np.float16)
    sink_ = np.ascontiguousarray(
        np.roll((sin_sel * kw * sign).T, 64, axis=0)).astype(np.float16)
    xTf = np.ascontiguousarray(x.reshape(S, HID).T).astype(np.float16)
    wq = np.asarray(wq, np.float32)
    wk = np.asarray(wk, np.float32)
    wv = np.asarray(wv, np.float32)
    wo = np.asarray(wo, np.float32)

    in_maps = []
    for c in range(N_CORES):
        wq_c = wq[c * G * P:(c + 1) * G * P, :].T        # [HID, 512]
        wk_c = wk[c * P:(c + 1) * P, :].T                # [HID, 128]
        wv_c = wv[c * P:(c + 1) * P, :].T                # [HID, 128]
        wqkv = np.concatenate([wq_c, wk_c, wv_c], axis=1)
        m = {
            "xT": xTf,
            "wqkvT": np.ascontiguousarray(wqkv).astype(np.float16),
            # 1/16 compensates the denominator rescale done on device
            # to keep the fp16 ones-matmul operand in range
            "woT": np.ascontiguousarray(
                wo[:, c * G * P:(c + 1) * G * P].T / 16.0).astype(np.float16),
            "cosq": cosq_, "sinq": sinq_, "cosk": cosk_, "sink": sink_,
        }
        in_maps.append(m)
    return in_maps


_NC = None


def _get_nc():
    global _NC
    if _NC is None:
        _NC = build_program()
    return _NC


def run_on_device(in_maps, trace=False):
    from concourse.bass_utils import run_bass_kernel_spmd
    nc = _get_nc()
    return run_bass_kernel_spmd(nc, in_maps, list(range(N_CORES)), trace=trace)


def kernel(**inputs):
    in_maps = shard_inputs(**inputs)
    res = run_on_device(in_maps).results
    y = np.zeros((S, HID), np.float32)
    for c in range(N_CORES):
        y += res[c]["y"].astype(np.float32)
    return y.reshape(1, S, HID)


# revision 47
# speedup vs baseline: 1.1040x; 1.1040x over previous
"""GQA attention prefill kernel for Trainium2 (Bass/Tile), 8-way tensor
parallel over heads.

Problem (hardcoded): B=1, S=2048, HID=4096, NH=32, KVH=8, D=128, causal
prefill with per-head RMSNorm on q/k and RoPE, positions = arange(S).

Sharding: core c owns kv-head c and q-heads 4c..4c+3. wq/wo sharded on the
head dim, wk/wv on the kv-head dim; x, rope tables replicated. Each core
computes its 4 heads' contribution through wo; the host sums the 8 partial
outputs (partials shipped as fp16, summed in fp32 on host).

Host-side prep (part of sharding): all matmul operands are fed pre-
transposed (contraction dim major) and pre-converted to fp16 — fp16
matmuls run at the same 1 cycle/row as fp32r but halve DMA and SBUF. The
q/k norm weights, the rotate-half sign, and the softmax 1/sqrt(d) are
folded into the rope tables; the sin tables are additionally pre-rotated
by 64 so the on-device rotate-half becomes a single PE matmul against a
fixed permutation matrix (no SBUF->SBUF DMAs).

Two on-device phases to keep PSUM-bank pressure per phase under the 8
banks (cross-phase overlap then never deadlocks the scheduler):
  1. per 512-column chunk: stream x once (quarter-chunk DMAs), project
     q/k/v (v directly transposed via x-as-stationary matmuls), rope +
     rmsnorm into resident KR / qr tiles. rsqrt is computed as
     exp(-0.5*ln(.)): with a manually preloaded combined {ln,exp,copy,
     square} activation table set the whole kernel needs exactly one
     ACT table load.
  2. causal attention per chunk (exp'd score tiles in bf16 — fp16 would
     overflow at score ~ +11.4; softmax denominators accumulate on the
     otherwise-idle Pool engine, one f32 ones-matmul per (head, chunk)
     folds the partitions) followed by the wo matmul; y partials are
     staged in fp16 and written by DMAs issued from the Pool queue so
     the SP queue never delays x prefetches.
"""

import numpy as np

import concourse.bass as bass
import concourse.mybir as mybir
import concourse.tile as tile
from concourse import bacc

P = 128
S = 2048
HID = 4096
D = 128
G = 4            # q heads per core
NHT = HID // P   # 32 h-tiles (contraction)
SC = 512         # seq chunk
NSC = S // SC    # 4
NKT = S // P     # 16 k-tiles
EPS = 1e-6
N_CORES = 8
WQKV = G * P + 2 * P   # 768 packed projection output dims (q|k|v)

F32 = mybir.dt.float32
F16 = mybir.dt.float16
BF16 = mybir.dt.bfloat16

ACT_SET_LN_EXP = 6     # natural_log_exp_and_others: {ln, exp, copy, square}


def build_program(dbg=False):
    nc = bacc.Bacc("TRN2", target_bir_lowering=False, debug=False)
    if dbg:
        KR_dbg = nc.dram_tensor("KR_dbg", [P, S], F16,
                                kind="ExternalOutput").ap()
        QR_dbg = nc.dram_tensor("QR_dbg", [P, G, S], F16,
                                kind="ExternalOutput").ap()
        V_dbg = nc.dram_tensor("V_dbg", [P, NKT, P], BF16,
                               kind="ExternalOutput").ap()
        OT_dbg = nc.dram_tensor("OT_dbg", [P, G, S], F16,
                                kind="ExternalOutput").ap()

    xT = nc.dram_tensor("xT", [HID, S], F16, kind="ExternalInput").ap()
    wqkvT = nc.dram_tensor("wqkvT", [HID, WQKV], F16, kind="ExternalInput").ap()
    woT = nc.dram_tensor("woT", [G * P, HID], F16, kind="ExternalInput").ap()
    cosq = nc.dram_tensor("cosq", [D, S], F16, kind="ExternalInput").ap()
    sinq = nc.dram_tensor("sinq", [D, S], F16, kind="ExternalInput").ap()
    cosk = nc.dram_tensor("cosk", [D, S], F16, kind="ExternalInput").ap()
    sink = nc.dram_tensor("sink", [D, S], F16, kind="ExternalInput").ap()
    y = nc.dram_tensor("y", [S, HID], F16, kind="ExternalOutput").ap()

    Ln = mybir.ActivationFunctionType.Ln
    Exp = mybir.ActivationFunctionType.Exp

    with tile.TileContext(nc) as tc:
        with (
            tc.tile_pool(name="const", bufs=1) as const,
            tc.tile_pool(name="tabs", bufs=1) as tabs,
            tc.tile_pool(name="xw", bufs=4) as xw,
            tc.tile_pool(name="scr", bufs=2) as scr,
            tc.tile_pool(name="rawp", bufs=6) as rawp,
            tc.tile_pool(name="otp", bufs=13) as otp,
            tc.tile_pool(name="ptp", bufs=5) as ptp,
            tc.tile_pool(name="dnp", bufs=4) as dnp,
            tc.tile_pool(name="yp", bufs=2) as yp,
            tc.tile_pool(name="ps", bufs=6, space="PSUM") as ps,
            tc.tile_pool(name="psd", bufs=2, space="PSUM") as psd,
        ):
            # one activation-table load for the whole kernel
            nc.scalar.add_instruction(mybir.InstLoadActFuncSet(
                name=nc.get_next_instruction_name(), ins=[], outs=[],
                act_func_set_id=ACT_SET_LN_EXP))

            # ---- input DMAs, interleaved so x quarters and the weight
            # parts feeding the first projections arrive together; the
            # later weight parts and woT go on the idle ACT queue ----
            xTr = xT.rearrange("(a p) s -> p a s", p=P)
            wqkv_sb = const.tile([P, NHT, WQKV], F16)
            wqkvTr = wqkvT.rearrange("(a p) c -> p a c", p=P)
            xs_first = []
            for qi in range(4):
                xs = xw.tile([P, 8, SC], F16, tag="xs")
                nc.sync.dma_start(xs, xTr[:, 8 * qi:8 * (qi + 1), 0:SC])
                xs_first.append(xs)
                nc.sync.dma_start(wqkv_sb[:, 8 * qi:8 * qi + 4, :],
                                  wqkvTr[:, 8 * qi:8 * qi + 4, :])
                nc.scalar.dma_start(wqkv_sb[:, 8 * qi + 4:8 * qi + 8, :],
                                    wqkvTr[:, 8 * qi + 4:8 * qi + 8, :])
            woT_sb = const.tile([P, G, HID], F16)
            nc.scalar.dma_start(
                woT_sb, woT.rearrange("(g p) h -> p g h", p=P))

            # ---- constants ----------------------------------------------
            f32tmp = const.tile([P, SC], F32)
            f32tmp2 = const.tile([P, P], F32)

            # ones[k, m] == 1: matmul(out, ones, rhs) -> column sums of rhs
            # broadcast across all 128 output partitions.
            ones16 = const.tile([P, P], F16)
            nc.gpsimd.memset(f32tmp, 1.0)
            nc.vector.tensor_copy(ones16, f32tmp[:, 0:P])
            ones32 = const.tile([P, P], F32)
            nc.gpsimd.memset(ones32, 1.0)

            # rotP: lhsT permutation with rotP[k, i] = 1 iff i == (k+64)%128,
            # so matmul(out, rotP, v) = v rotated-half along partitions.
            nc.gpsimd.memset(f32tmp2, 1.0)
            rsel1 = const.tile([P, P], F32)
            nc.gpsimd.affine_select(
                rsel1, f32tmp2, pattern=[[1, P]],
                compare_op=mybir.AluOpType.is_equal,
                fill=0.0, base=-64, channel_multiplier=-1,
            )
            rsel2 = const.tile([P, P], F32)
            nc.gpsimd.affine_select(
                rsel2, f32tmp2, pattern=[[1, P]],
                compare_op=mybir.AluOpType.is_equal,
                fill=0.0, base=64, channel_multiplier=-1,
            )
            nc.vector.tensor_add(rsel1, rsel1, rsel2)
            rotP = const.tile([P, P], F16)
            nc.vector.tensor_copy(rotP, rsel1)

            # lower-triangular causal mask for the exact-diagonal 128x128
            # tiles: keep (1.0) where q_local >= k_local
            nc.gpsimd.memset(f32tmp2, 1.0)
            trisel = const.tile([P, P], F32)
            nc.gpsimd.affine_select(
                trisel, f32tmp2, pattern=[[1, P]],
                compare_op=mybir.AluOpType.is_ge,
                fill=0.0, base=0, channel_multiplier=-1,
            )
            trimask = const.tile([P, P], BF16)
            nc.vector.tensor_copy(trimask, trisel)

            bias_keps = const.tile([P, 1], F32)
            nc.gpsimd.memset(bias_keps, float(P) * EPS)
            bias_qeps = const.tile([P, 1], F32)
            nc.gpsimd.memset(bias_qeps, EPS)

            # ---- resident tensors ---------------------------------------
            KR = const.tile([P, S], F16)          # roped+scaled K, [d, s]
            Vs = const.tile([P, NKT, P], BF16)    # V, [s-in-tile, k-tile, d]
            qr_all = const.tile([P, G, S], F16)   # roped+scaled Q, [d, h, s]

            # ============ per-chunk emitters ============================
            def rope_one(zraw, cos_t, sin_t, out_ap, bias_ap, ln_scale):
                sq = scr.tile([P, SC], F16, tag="sq")
                nc.vector.tensor_mul(sq, zraw, zraw)
                ssb = ps.tile([P, SC], F32, tag="ps")
                nc.tensor.matmul(ssb, ones16, sq, start=True, stop=True)
                # 1/sqrt(ssq + d*eps) == exp(-0.5*ln(ssq + d*eps)):
                # the rms-norm (and for K the softmax 1/sqrt(d)) in one go
                ln_ = scr.tile([P, SC], F32, tag="lnk")
                nc.scalar.activation(ln_, ssb, Ln, bias=bias_ap,
                                     scale=ln_scale)
                rf = scr.tile([P, SC], F32, tag="rk")
                nc.scalar.activation(rf, ln_, Exp, bias=0.0, scale=-0.5)
                # rope: out = z*cos + rot(z*sin_pre), sin pre-rotated on host
                t1 = scr.tile([P, SC], F16, tag="t1")
                nc.vector.tensor_mul(t1, zraw, sin_t)
                rps = ps.tile([P, SC], F32, tag="ps")
                nc.tensor.matmul(rps, rotP, t1, start=True, stop=True)
                pre = scr.tile([P, SC], F32, tag="kpre")
                nc.vector.tensor_mul(pre, zraw, cos_t)
                nc.vector.tensor_add(pre, pre, rps)
                nc.vector.tensor_mul(out_ap, pre, rf)

            def proj_rope(sc):
                q0 = sc * SC
                # x streamed in 4 quarter-chunk tiles of 8 h-tiles each;
                # q projection runs in two passes of 2 heads to keep at
                # most 3 projection PSUM banks live at a time
                xss = []
                qps = [None] * G
                qps[0] = ps.tile([P, SC], F32, tag="ps", name="qps0")
                qps[1] = ps.tile([P, SC], F32, tag="ps", name="qps1")
                kps = ps.tile([P, SC], F32, tag="ps")
                for qi in range(4):
                    if sc == 0:
                        xs = xs_first[qi]
                    else:
                        xs = xw.tile([P, 8, SC], F16, tag="xs")
                        nc.sync.dma_start(
                            xs, xTr[:, 8 * qi:8 * (qi + 1), q0:q0 + SC])
                    xss.append(xs)
                    for ht8 in range(8):
                        ht = qi * 8 + ht8
                        xt = xs[:, ht8, :]
                        st = ht == 0
                        sp = ht == NHT - 1
                        for mt in range(2):
                            nc.tensor.matmul(
                                qps[mt],
                                wqkv_sb[:, ht, mt * P:(mt + 1) * P], xt,
                                start=st, stop=sp,
                            )
                        nc.tensor.matmul(
                            kps, wqkv_sb[:, ht, G * P:G * P + P], xt,
                            start=st, stop=sp,
                        )

                cq = tabs.tile([P, SC], F16, tag="cosq")
                nc.sync.dma_start(cq, cosq[:, q0:q0 + SC])
                sq_ = tabs.tile([P, SC], F16, tag="sinq")
                nc.sync.dma_start(sq_, sinq[:, q0:q0 + SC])
                ck = tabs.tile([P, SC], F16, tag="cosk")
                nc.sync.dma_start(ck, cosk[:, q0:q0 + SC])
                sk = tabs.tile([P, SC], F16, tag="sink")
                nc.sync.dma_start(sk, sink[:, q0:q0 + SC])

                # K rope (and q0/q1 copies) free pass-A banks while the
                # pass-B / v matmuls keep the PE busy
                kraw = rawp.tile([P, SC], F16, tag="raw")
                nc.scalar.copy(kraw, kps)
                qraws = [None] * G
                for h in range(2):
                    qraw = rawp.tile([P, SC], F16, tag="raw")
                    nc.scalar.copy(qraw, qps[h])
                    qraws[h] = qraw
                rope_one(kraw, ck, sk, KR[:, q0:q0 + SC],
                         bias_keps, 1.0)

                # pass B: q heads 2,3
                qps[2] = ps.tile([P, SC], F32, tag="ps", name="qps2")
                qps[3] = ps.tile([P, SC], F32, tag="ps", name="qps3")
                for ht in range(NHT):
                    xt = xss[ht // 8][:, ht % 8, :]
                    st = ht == 0
                    sp = ht == NHT - 1
                    for mt in range(2, G):
                        nc.tensor.matmul(
                            qps[mt], wqkv_sb[:, ht, mt * P:(mt + 1) * P],
                            xt, start=st, stop=sp,
                        )

                # v, directly transposed: out[s, d]; each 128-row s-subtile
                # accumulates in its own small PSUM tile (a PSUM bank region
                # only supports one accumulation group at a time)
                for j in range(4):
                    vpsd = psd.tile([P, P], F32, tag="psq")
                    for ht in range(NHT):
                        nc.tensor.matmul(
                            vpsd,
                            xss[ht // 8][:, ht % 8, j * P:(j + 1) * P],
                            wqkv_sb[:, ht, G * P + P:],
                            start=(ht == 0), stop=(ht == NHT - 1),
                        )
                    nc.scalar.copy(Vs[:, sc * 4 + j, :], vpsd)

                for h in range(2, G):
                    qraw = rawp.tile([P, SC], F16, tag="raw")
                    nc.scalar.copy(qraw, qps[h])
                    qraws[h] = qraw
                for h in range(2):
                    rope_one(qraws[h], cq, sq_,
                             qr_all[:, h, q0:q0 + SC], bias_qeps, 1.0 / P)
                for h in range(2, G):
                    rope_one(qraws[h], cq, sq_,
                             qr_all[:, h, q0:q0 + SC], bias_qeps, 1.0 / P)

            def attn(sc):
                q0 = sc * SC
                ots = []
                off = sc * 4          # full (off-diagonal) k-tiles
                nkt = off + 4
                for h in range(G):
                    qrh = qr_all[:, h, q0:q0 + SC]
                    avp = ps.tile([P, SC], F32, tag="ps")
                    # two denominator accumulators: even k-tiles on Pool,
                    # odd on DVE — two independent serial chains
                    dnacc = [dnp.tile([P, SC], F32, tag="dn", name=f"dn{i}")
                             for i in range(2)]
                    if off == 0:
                        # first chunk: diagonal rectangles don't cover the
                        # accumulators' full width, so zero-init them
                        nc.gpsimd.memset(dnacc[0], 0.0)
                        nc.vector.memset(dnacc[1], 0.0)
                    for kt in range(nkt):
                        # diagonal k-tile kt2 only attends q columns
                        # >= 128*kt2: compute the valid rectangle only
                        c0 = (kt - off) * P if kt >= off else 0
                        ptps = ps.tile([P, SC], F32, tag="ps")
                        nc.tensor.matmul(
                            ptps[:, c0:], KR[:, kt * P:(kt + 1) * P],
                            qrh[:, c0:],
                            start=True, stop=True,
                        )
                        pt = ptp.tile([P, SC], BF16, tag="pt")
                        nc.scalar.activation(pt[:, c0:], ptps[:, c0:],
                                             Exp, bias=0.0, scale=1.0)
                        if kt >= off:
                            nc.vector.tensor_mul(pt[:, c0:c0 + P],
                                                 pt[:, c0:c0 + P], trimask)
                        eng = nc.gpsimd if kt % 2 == 0 else nc.vector
                        acc = dnacc[kt % 2]
                        if kt < 2 and off > 0:
                            eng.tensor_copy(acc, pt)
                        else:
                            eng.tensor_add(acc[:, c0:], acc[:, c0:],
                                           pt[:, c0:])
                        nc.tensor.matmul(avp[:, c0:], Vs[:, kt, :],
                                         pt[:, c0:],
                                         start=(kt == 0),
                                         stop=(kt == nkt - 1))
                    # fold the 128 partition-partials of the denominator;
                    # 1/16 rescale (fp16 headroom) is folded into woT on host
                    dnps = ps.tile([P, SC], F32, tag="ps")
                    for i, acc in enumerate(dnacc):
                        dn16 = scr.tile([P, SC], F16, tag="dn16")
                        nc.gpsimd.tensor_scalar_mul(dn16, acc, 1.0 / 16.0)
                        nc.tensor.matmul(dnps, ones16, dn16,
                                         start=(i == 0),
                                         stop=(i == len(dnacc) - 1))
                    rcp = scr.tile([P, SC], F32, tag="lnk")
                    nc.vector.reciprocal(rcp, dnps)
                    ot = otp.tile([P, SC], F16, tag="ot")
                    nc.vector.tensor_mul(ot, avp, rcp)
                    ots.append(ot)
                    if dbg:
                        nc.gpsimd.dma_start(OT_dbg[:, h, q0:q0 + SC], ot)
                return ots

            def wo_proj(sc, ots):
                q0 = sc * SC
                # output projection: 4 column groups of 2 PSUM banks
                for stl in range(SC // P):
                    srow = q0 + stl * P
                    for grp in range(4):
                        ybuf = yp.tile([P, 2 * SC], F16, tag="ys")
                        yps_l = [ps.tile([P, SC], F32, tag="ps",
                                         name=f"yps{j}")
                                 for j in range(2)]
                        for h in range(G):
                            lhs = ots[h][:, stl * P:(stl + 1) * P]
                            for j in range(2):
                                hc = grp * 2 + j
                                nc.tensor.matmul(
                                    yps_l[j], lhs,
                                    woT_sb[:, h, hc * SC:(hc + 1) * SC],
                                    start=(h == 0), stop=(h == G - 1),
                                )
                        nc.scalar.copy(ybuf[:, 0:SC], yps_l[0])
                        nc.vector.tensor_copy(ybuf[:, SC:2 * SC], yps_l[1])
                        nc.gpsimd.dma_start(
                            y[srow:srow + P,
                              grp * 2 * SC:(grp + 1) * 2 * SC], ybuf)

            # software-pipelined emission: attention of chunk sc is emitted
            # before projections of chunk sc+1, so the scheduler prefers
            # the latency-critical attention chain and fills its ACT-bound
            # gaps with projection matmuls. Chunk 0's wo is deferred to the
            # very end, where it fills the exp-bound gaps of the last
            # chunk's attention (which has no projection work left to
            # overlap with).
            proj_rope(0)
            ots0 = attn(0)
            proj_rope(1)
            for sc in range(1, NSC - 1):
                ots_sc = attn(sc)
                wo_proj(sc, ots_sc)
                proj_rope(sc + 1)
            ots3 = attn(NSC - 1)
            wo_proj(0, ots0)
            wo_proj(NSC - 1, ots3)

            if dbg:
                nc.sync.dma_start(KR_dbg, KR)
                nc.sync.dma_start(QR_dbg, qr_all)
                nc.sync.dma_start(V_dbg, Vs)

    nc.finalize()
    return nc


def shard_inputs(x, wq, wk, wv, wo, q_norm_w, k_norm_w, cos_table, sin_table,
                 positions, **_ignored):
    """Host-side sharding: returns the list of 8 per-core input maps."""
    x = np.asarray(x, np.float32)
    pos = np.asarray(positions).astype(np.int64)
    cos_sel = np.asarray(cos_table, np.float32)[pos]   # [S, D]
    sin_sel = np.asarray(sin_table, np.float32)[pos]
    qw = np.asarray(q_norm_w, np.float32)
    kw = np.asarray(k_norm_w, np.float32)
    # fold norm weights into the transposed rope tables:
    # w * rope(q') == q'*(w*cos) + rot(q')*(w*sin)
    # fold rotate-half's minus sign into sin rows 0..63:
    # rope(z) = z*cos + [-z2; z1]*sin = z*cos + rot(z)*sin_eff
    # and pre-rotate sin so rot(z)*sin_eff == rot(z * rot(sin_eff))
    sign = np.ones((1, D), np.float32)
    sign[0, :D // 2] = -1.0
    cosq_ = np.ascontiguousarray((cos_sel * qw).T).astype(np.float16)
    sinq_ = np.ascontiguousarray(
        np.roll((sin_sel * qw * sign).T, 64, axis=0)).astype(np.float16)
    cosk_ = np.ascontiguousarray((cos_sel * kw).T).astype(np.float16)
    sink_ = np.ascontiguousarray(
        np.roll((sin_sel * kw * sign).T, 64, axis=0)).astype(np.float16)
    xTf = np.ascontiguousarray(x.reshape(S, HID).T).astype(np.float16)
    wq = np.asarray(wq, np.float32)
    wk = np.asarray(wk, np.float32)
    wv = np.asarray(wv, np.float32)
    wo = np.asarray(wo, np.float32)

    in_maps = []
    for c in range(N_CORES):
        wq_c = wq[c * G * P:(c + 1) * G * P, :].T        # [HID, 512]
        wk_c = wk[c * P:(c + 1) * P, :].T                # [HID, 128]
        wv_c = wv[c * P:(c + 1) * P, :].T                # [HID, 128]
        wqkv = np.concatenate([wq_c, wk_c, wv_c], axis=1)
        m = {
            "xT": xTf,
            "wqkvT": np.ascontiguousarray(wqkv).astype(np.float16),
            # 1/16 compensates the denominator rescale done on device
            # to keep the fp16 ones-matmul operand in range
            "woT": np.ascontiguousarray(
                wo[:, c * G * P:(c + 1) * G * P].T / 16.0).astype(np.float16),
            "cosq": cosq_, "sinq": sinq_, "cosk": cosk_, "sink": sink_,
        }
        in_maps.append(m)
    return in_maps


_NC = None


def _get_nc():
    global _NC
    if _NC is None:
        _NC = build_program()
    return _NC


def run_on_device(in_maps, trace=False):
    from concourse.bass_utils import run_bass_kernel_spmd
    nc = _get_nc()
    return run_bass_kernel_spmd(nc, in_maps, list(range(N_CORES)), trace=trace)


def kernel(**inputs):
    in_maps = shard_inputs(**inputs)
    res = run_on_device(in_maps).results
    y = np.zeros((S, HID), np.float32)
    for c in range(N_CORES):
        y += res[c]["y"].astype(np.float32)
    return y.reshape(1, S, HID)


# revision 119
# speedup vs baseline: 1.2201x; 1.1052x over previous
"""GQA attention prefill kernel for Trainium2 (Bass/Tile), 8-way tensor
parallel over heads.

Problem (hardcoded): B=1, S=2048, HID=4096, NH=32, KVH=8, D=128, causal
prefill with per-head RMSNorm on q/k and RoPE, positions = arange(S).

Sharding: core c owns kv-head c and q-heads 4c..4c+3. wq/wo sharded on the
head dim, wk/wv on the kv-head dim; x, rope tables replicated. Each core
computes its 4 heads' contribution through wo; the host sums the 8 partial
outputs (partials shipped as fp16, summed in fp32 on host).

Host-side prep (part of sharding): all matmul operands are fed pre-
transposed (contraction dim major) and pre-converted to fp16 — fp16
matmuls run at the same 1 cycle/row as fp32r but halve DMA and SBUF. The
q/k norm weights, the rotate-half sign, and the softmax 1/sqrt(d) are
folded into the rope tables; the sin tables are additionally pre-rotated
by 64 so the on-device rotate-half becomes a single PE matmul against a
fixed permutation matrix (no SBUF->SBUF DMAs).

On-device schedule: one software-pipelined stream per 512-column seq
chunk — projections+rope of chunk sc+1 are emitted after attention of
chunk sc so the scheduler fills the exp-rate-limited attention windows
with projection matmuls (and vice versa fills projection stalls with
attention/wo work). Details:
  - projections: stream x once (quarter-chunk DMAs); q heads in two
    2-head passes and v in per-subtile PSUM tiles to bound PSUM bank
    pressure (a PSUM bank region supports one accumulation group at a
    time). v is produced directly transposed via x-as-stationary
    matmuls.
  - rope+rmsnorm into resident KR / qr tiles; rsqrt is computed as
    exp(-0.5*ln(.)): with a manually preloaded combined {ln,exp,copy,
    square} activation table set the whole kernel needs exactly one
    ACT table load.
  - causal attention computes only the valid rectangle of each
    diagonal k-tile (partial-width PSUM accumulation); the causal mask
    is an additive -30000 upper-triangular constant applied in PSUM by
    a tiny identity-matmul, so nothing sits between the exp and the AV
    matmul. Score exponentials are computed as exp(s - ln16): rmsnorm
    bounds scores at sqrt(128)=11.31, so exp values stay under 5.2e3
    and fit fp16 (the 1/16 cancels between numerator and denominator).
    Denominators accumulate in two alternating fp16 chains on the Pool
    and DVE engines (all-2-byte operands hit the DVE 2x mode); one
    fp16 ones-matmul per (head, chunk) folds the partitions.
  - wo matmul in 2-bank column groups; y partials staged in fp16 and
    written by DMAs issued from the Pool queue so the SP queue never
    delays x prefetches.
PSUM banks are statically split 4 (projections/rope/attention-out/wo)
+ 3 (score tiles, the latency-critical pipeline) + 1 (v subtiles).
"""

import numpy as np

import concourse.bass as bass
import concourse.mybir as mybir
import concourse.tile as tile
from concourse import bacc

P = 128
S = 2048
HID = 4096
D = 128
G = 4            # q heads per core
NHT = HID // P   # 32 h-tiles (contraction)
SC = 512         # seq chunk
NSC = S // SC    # 4
NKT = S // P     # 16 k-tiles
EPS = 1e-6
N_CORES = 8
WQKV = G * P + 2 * P   # 768 packed projection output dims (q|k|v)

F32 = mybir.dt.float32
F16 = mybir.dt.float16
BF16 = mybir.dt.bfloat16

ACT_SET_LN_EXP = 6     # natural_log_exp_and_others: {ln, exp, copy, square}


def build_program(dbg=False):
    nc = bacc.Bacc("TRN2", target_bir_lowering=False, debug=False)
    if dbg:
        KR_dbg = nc.dram_tensor("KR_dbg", [P, S], F16,
                                kind="ExternalOutput").ap()
        QR_dbg = nc.dram_tensor("QR_dbg", [P, G, S], F16,
                                kind="ExternalOutput").ap()
        V_dbg = nc.dram_tensor("V_dbg", [P, NKT, P], BF16,
                               kind="ExternalOutput").ap()
        OT_dbg = nc.dram_tensor("OT_dbg", [P, G, S], F16,
                                kind="ExternalOutput").ap()

    xT = nc.dram_tensor("xT", [HID, S], F16, kind="ExternalInput").ap()
    wqkvT = nc.dram_tensor("wqkvT", [HID, WQKV], F16, kind="ExternalInput").ap()
    woT = nc.dram_tensor("woT", [G * P, HID], F16, kind="ExternalInput").ap()
    cosq = nc.dram_tensor("cosq", [D, S], F16, kind="ExternalInput").ap()
    sinq = nc.dram_tensor("sinq", [D, S], F16, kind="ExternalInput").ap()
    cosk = nc.dram_tensor("cosk", [D, S], F16, kind="ExternalInput").ap()
    sink = nc.dram_tensor("sink", [D, S], F16, kind="ExternalInput").ap()
    y = nc.dram_tensor("y", [S, HID], F16, kind="ExternalOutput").ap()

    Ln = mybir.ActivationFunctionType.Ln
    Exp = mybir.ActivationFunctionType.Exp

    with tile.TileContext(nc) as tc:
        with (
            tc.tile_pool(name="const", bufs=1) as const,
            tc.tile_pool(name="tabs", bufs=2) as tabs,
            tc.tile_pool(name="xw", bufs=4) as xw,
            tc.tile_pool(name="scr", bufs=2) as scr,
            tc.tile_pool(name="rawp", bufs=6) as rawp,
            tc.tile_pool(name="otp", bufs=13) as otp,
            tc.tile_pool(name="ptp", bufs=5) as ptp,
            tc.tile_pool(name="dnp", bufs=7) as dnp,
            tc.tile_pool(name="yp", bufs=2) as yp,
            tc.tile_pool(name="ps", bufs=4, space="PSUM") as ps,
            tc.tile_pool(name="pts", bufs=3, space="PSUM") as pts,
            tc.tile_pool(name="psd", bufs=1, space="PSUM") as psd,
        ):
            # one activation-table load for the whole kernel
            nc.scalar.add_instruction(mybir.InstLoadActFuncSet(
                name=nc.get_next_instruction_name(), ins=[], outs=[],
                act_func_set_id=ACT_SET_LN_EXP))

            # ---- input DMAs, interleaved so x quarters and the weight
            # parts feeding the first projections arrive together; the
            # later weight parts and woT go on the idle ACT queue ----
            xTr = xT.rearrange("(a p) s -> p a s", p=P)
            wqkv_sb = const.tile([P, NHT, WQKV], F16)
            wqkvTr = wqkvT.rearrange("(a p) c -> p a c", p=P)
            xs_first = []
            for qi in range(4):
                xs = xw.tile([P, 8, SC], F16, tag="xs")
                if qi == 0:
                    # first h-tile arrives via the otherwise-idle ACT queue
                    # so the very first projection matmul starts ~1.5us in
                    nc.scalar.dma_start(xs[:, 0:1, :], xTr[:, 0:1, 0:SC])
                    nc.scalar.dma_start(wqkv_sb[:, 0:1, :],
                                        wqkvTr[:, 0:1, :])
                    nc.sync.dma_start(xs[:, 1:8, :], xTr[:, 1:8, 0:SC])
                    nc.sync.dma_start(wqkv_sb[:, 1:4, :], wqkvTr[:, 1:4, :])
                else:
                    nc.sync.dma_start(xs, xTr[:, 8 * qi:8 * (qi + 1), 0:SC])
                    nc.sync.dma_start(wqkv_sb[:, 8 * qi:8 * qi + 4, :],
                                      wqkvTr[:, 8 * qi:8 * qi + 4, :])
                xs_first.append(xs)
                nc.scalar.dma_start(wqkv_sb[:, 8 * qi + 4:8 * qi + 8, :],
                                    wqkvTr[:, 8 * qi + 4:8 * qi + 8, :])
            woT_sb = const.tile([P, G, HID], F16)

            # ---- constants ----------------------------------------------
            f32tmp = const.tile([P, SC], F32)
            f32tmp2 = const.tile([P, P], F32)

            # ones[k, m] == 1: matmul(out, ones, rhs) -> column sums of rhs
            # broadcast across all 128 output partitions.
            ones16 = const.tile([P, P], F16)
            nc.gpsimd.memset(f32tmp, 1.0)
            nc.vector.tensor_copy(ones16, f32tmp[:, 0:P])

            # rotP: lhsT permutation with rotP[k, i] = 1 iff i == (k+64)%128,
            # so matmul(out, rotP, v) = v rotated-half along partitions.
            nc.gpsimd.memset(f32tmp2, 1.0)
            rsel1 = const.tile([P, P], F32)
            nc.gpsimd.affine_select(
                rsel1, f32tmp2, pattern=[[1, P]],
                compare_op=mybir.AluOpType.is_equal,
                fill=0.0, base=-64, channel_multiplier=-1,
            )
            rsel2 = const.tile([P, P], F32)
            nc.gpsimd.affine_select(
                rsel2, f32tmp2, pattern=[[1, P]],
                compare_op=mybir.AluOpType.is_equal,
                fill=0.0, base=64, channel_multiplier=-1,
            )
            nc.vector.tensor_add(rsel1, rsel1, rsel2)
            rotP = const.tile([P, P], F16)
            nc.vector.tensor_copy(rotP, rsel1)

            # additive causal mask for the exact-diagonal 128x128 tiles:
            # -30000 where q_local < k_local (exp -> 0), else 0. Applied in
            # PSUM via matmul(identity, umask) accumulation so no vector op
            # sits between the exp and the AV matmul.
            ident16 = const.tile([P, P], F16)
            nc.gpsimd.memset(f32tmp2, 1.0)
            isel = const.tile([P, P], F32)
            nc.gpsimd.affine_select(
                isel, f32tmp2, pattern=[[1, P]],
                compare_op=mybir.AluOpType.is_equal,
                fill=0.0, base=0, channel_multiplier=-1,
            )
            nc.vector.tensor_copy(ident16, isel)
            nc.gpsimd.memset(f32tmp2, -30000.0)
            usel = const.tile([P, P], F32)
            nc.gpsimd.affine_select(
                usel, f32tmp2, pattern=[[-1, P]],
                compare_op=mybir.AluOpType.is_ge,
                fill=0.0, base=-1, channel_multiplier=1,
            )
            umask16 = const.tile([P, P], F16)
            nc.vector.tensor_copy(umask16, usel)

            bias_keps = const.tile([P, 1], F32)
            nc.gpsimd.memset(bias_keps, float(P) * EPS)
            bias_qeps = const.tile([P, 1], F32)
            nc.gpsimd.memset(bias_qeps, EPS)
            # exp(s - ln16): keeps the fp16 softmax-denominator
            # accumulators in range; cancels between numerator/denominator
            bias_ln16 = const.tile([P, 1], F32)
            nc.gpsimd.memset(bias_ln16, -2.7725887)

            # ---- resident tensors ---------------------------------------
            KR = const.tile([P, S], F16)          # roped+scaled K, [d, s]
            Vs = const.tile([P, NKT, P], BF16)    # V, [s-in-tile, k-tile, d]
            qr_all = const.tile([P, G, S], F16)   # roped+scaled Q, [d, h, s]

            # ============ per-chunk emitters ============================
            def rope_one(zraw, cos_t, sin_t, out_ap, bias_ap, ln_scale):
                sq = scr.tile([P, SC], F16, tag="sq")
                nc.vector.tensor_mul(sq, zraw, zraw)
                ssb = pts.tile([P, SC], F32, tag="pts")
                nc.tensor.matmul(ssb, ones16, sq, start=True, stop=True)
                # 1/sqrt(ssq + d*eps) == exp(-0.5*ln(ssq + d*eps)):
                # the rms-norm (and for K the softmax 1/sqrt(d)) in one go
                ln_ = scr.tile([P, SC], F32, tag="lnk")
                nc.scalar.activation(ln_, ssb, Ln, bias=bias_ap,
                                     scale=ln_scale)
                rf = scr.tile([P, SC], F32, tag="rk")
                nc.scalar.activation(rf, ln_, Exp, bias=0.0, scale=-0.5)
                # rope: out = z*cos + rot(z*sin_pre), sin pre-rotated on
                # host; the two SBUF-only muls run on the idle Pool engine
                # so the DVE chain (which must touch PSUM) stays short
                t1 = scr.tile([P, SC], F16, tag="t1")
                nc.gpsimd.tensor_mul(t1, zraw, sin_t)
                rps = pts.tile([P, SC], F32, tag="pts")
                nc.tensor.matmul(rps, rotP, t1, start=True, stop=True)
                pre = scr.tile([P, SC], F32, tag="kpre")
                nc.gpsimd.tensor_mul(pre, zraw, cos_t)
                nc.vector.tensor_add(pre, pre, rps)
                nc.vector.tensor_mul(out_ap, pre, rf)

            def proj_rope(sc):
                q0 = sc * SC
                # x streamed in 4 quarter-chunk tiles of 8 h-tiles each;
                # q projection runs in two passes of 2 heads to keep at
                # most 3 projection PSUM banks live at a time
                xss = []
                qps = [None] * G
                qps[0] = ps.tile([P, SC], F32, tag="ps", name="qps0")
                qps[1] = ps.tile([P, SC], F32, tag="ps", name="qps1")
                kps = ps.tile([P, SC], F32, tag="ps")
                for qi in range(4):
                    if sc == 0:
                        xs = xs_first[qi]
                    else:
                        xs = xw.tile([P, 8, SC], F16, tag="xs")
                        nc.sync.dma_start(
                            xs, xTr[:, 8 * qi:8 * (qi + 1), q0:q0 + SC])
                    xss.append(xs)
                    for ht8 in range(8):
                        ht = qi * 8 + ht8
                        xt = xs[:, ht8, :]
                        st = ht == 0
                        sp = ht == NHT - 1
                        for mt in range(2):
                            nc.tensor.matmul(
                                qps[mt],
                                wqkv_sb[:, ht, mt * P:(mt + 1) * P], xt,
                                start=st, stop=sp,
                            )
                        nc.tensor.matmul(
                            kps, wqkv_sb[:, ht, G * P:G * P + P], xt,
                            start=st, stop=sp,
                        )

                cq = tabs.tile([P, SC], F16, tag="cosq")
                nc.sync.dma_start(cq, cosq[:, q0:q0 + SC])
                sq_ = tabs.tile([P, SC], F16, tag="sinq")
                nc.sync.dma_start(sq_, sinq[:, q0:q0 + SC])
                ck = tabs.tile([P, SC], F16, tag="cosk")
                nc.sync.dma_start(ck, cosk[:, q0:q0 + SC])
                sk = tabs.tile([P, SC], F16, tag="sink")
                nc.sync.dma_start(sk, sink[:, q0:q0 + SC])

                # PSUM-freeing copies first; for sc>0 the q0/q1 ropes run
                # before the K rope so this chunk's attention (whose
                # off-diagonal tiles only need earlier chunks' KR) can
                # start as early as possible — the K rope only gates the
                # diagonal tiles, which come last in the k-tile order
                kraw = rawp.tile([P, SC], F16, tag="raw")
                nc.scalar.copy(kraw, kps)
                qraws = [None] * G
                for h in range(2):
                    qraw = rawp.tile([P, SC], F16, tag="raw")
                    nc.scalar.copy(qraw, qps[h])
                    qraws[h] = qraw
                if sc == 0:
                    rope_one(kraw, ck, sk, KR[:, q0:q0 + SC],
                             bias_keps, 1.0)
                else:
                    rope_one(qraws[0], cq, sq_,
                             qr_all[:, 0, q0:q0 + SC], bias_qeps, 1.0 / P)
                    rope_one(qraws[1], cq, sq_,
                             qr_all[:, 1, q0:q0 + SC], bias_qeps, 1.0 / P)

                # pass B: q heads 2,3
                qps[2] = ps.tile([P, SC], F32, tag="ps", name="qps2")
                qps[3] = ps.tile([P, SC], F32, tag="ps", name="qps3")
                for ht in range(NHT):
                    xt = xss[ht // 8][:, ht % 8, :]
                    st = ht == 0
                    sp = ht == NHT - 1
                    for mt in range(2, G):
                        nc.tensor.matmul(
                            qps[mt], wqkv_sb[:, ht, mt * P:(mt + 1) * P],
                            xt, start=st, stop=sp,
                        )

                # v, directly transposed: out[s, d]; each 128-row s-subtile
                # accumulates in its own small PSUM tile (a PSUM bank region
                # only supports one accumulation group at a time)
                for j in range(4):
                    vpsd = psd.tile([P, P], F32, tag="psq")
                    for ht in range(NHT):
                        nc.tensor.matmul(
                            vpsd,
                            xss[ht // 8][:, ht % 8, j * P:(j + 1) * P],
                            wqkv_sb[:, ht, G * P + P:],
                            start=(ht == 0), stop=(ht == NHT - 1),
                        )
                    nc.scalar.copy(Vs[:, sc * 4 + j, :], vpsd)

                for h in range(2, G):
                    qraw = rawp.tile([P, SC], F16, tag="raw")
                    nc.scalar.copy(qraw, qps[h])
                    qraws[h] = qraw
                if sc == 0:
                    for h in range(2):
                        rope_one(qraws[h], cq, sq_,
                                 qr_all[:, h, q0:q0 + SC], bias_qeps, 1.0 / P)
                for h in range(2, G):
                    rope_one(qraws[h], cq, sq_,
                             qr_all[:, h, q0:q0 + SC], bias_qeps, 1.0 / P)
                if sc > 0:
                    rope_one(kraw, ck, sk, KR[:, q0:q0 + SC],
                             bias_keps, 1.0)

            def attn(sc):
                q0 = sc * SC
                ots = []
                off = sc * 4          # full (off-diagonal) k-tiles
                nkt = off + 4
                for h in range(G):
                    qrh = qr_all[:, h, q0:q0 + SC]
                    avp = ps.tile([P, SC], F32, tag="ps")
                    # two denominator accumulators: even k-tiles on Pool,
                    # odd on DVE — two independent serial chains. exp is
                    # computed as exp(s - ln16), which bounds the fp16
                    # accumulators (the 1/16 cancels between numerator and
                    # denominator); all-2-byte operands let the DVE chain
                    # run in its 2x mode
                    dnacc = [dnp.tile([P, SC], F16, tag="dn", name=f"dn{i}")
                             for i in range(2)]
                    if off == 0:
                        # first chunk: diagonal rectangles don't cover the
                        # accumulators' full width, so zero-init them
                        nc.gpsimd.memset(dnacc[0], 0.0)
                        nc.vector.memset(dnacc[1], 0.0)
                    for kt in range(nkt):
                        # diagonal k-tile kt2 only attends q columns
                        # >= 128*kt2: compute the valid rectangle only
                        c0 = (kt - off) * P if kt >= off else 0
                        ptps = pts.tile([P, SC], F32, tag="pts")
                        diag = kt >= off
                        nc.tensor.matmul(
                            ptps[:, c0:], KR[:, kt * P:(kt + 1) * P],
                            qrh[:, c0:],
                            start=True, stop=not diag,
                        )
                        if diag:
                            # additive causal mask for the leading 128 q
                            # columns (the exact-diagonal subtile)
                            nc.tensor.matmul(
                                ptps[:, c0:c0 + P], ident16, umask16,
                                start=False, stop=True,
                            )
                        pt = ptp.tile([P, SC], BF16, tag="pt")
                        nc.scalar.activation(pt[:, c0:], ptps[:, c0:],
                                             Exp, bias=bias_ln16, scale=1.0)
                        eng = nc.gpsimd if kt % 2 == 0 else nc.vector
                        acc = dnacc[kt % 2]
                        if kt < 2 and off > 0:
                            eng.tensor_copy(acc, pt)
                        else:
                            eng.tensor_add(acc[:, c0:], acc[:, c0:],
                                           pt[:, c0:])
                        nc.tensor.matmul(avp[:, c0:], Vs[:, kt, :],
                                         pt[:, c0:],
                                         start=(kt == 0),
                                         stop=(kt == nkt - 1))
                    # fold the 128 partition-partials of the denominator
                    dnps = ps.tile([P, SC], F32, tag="ps")
                    for i, acc in enumerate(dnacc):
                        nc.tensor.matmul(dnps, ones16, acc,
                                         start=(i == 0),
                                         stop=(i == len(dnacc) - 1))
                    rcp = scr.tile([P, SC], F32, tag="rcp")
                    nc.vector.reciprocal(rcp, dnps)
                    ot = otp.tile([P, SC], F16, tag="ot")
                    nc.vector.tensor_mul(ot, avp, rcp)
                    ots.append(ot)
                    if dbg:
                        nc.gpsimd.dma_start(OT_dbg[:, h, q0:q0 + SC], ot)
                return ots

            def wo_proj(sc, ots):
                q0 = sc * SC
                # output projection: 4 column groups of 2 PSUM banks
                for stl in range(SC // P):
                    srow = q0 + stl * P
                    for grp in range(4):
                        ybuf = yp.tile([P, 2 * SC], F16, tag="ys")
                        yps_l = [ps.tile([P, SC], F32, tag="ps",
                                         name=f"yps{j}")
                                 for j in range(2)]
                        for h in range(G):
                            lhs = ots[h][:, stl * P:(stl + 1) * P]
                            for j in range(2):
                                hc = grp * 2 + j
                                nc.tensor.matmul(
                                    yps_l[j], lhs,
                                    woT_sb[:, h, hc * SC:(hc + 1) * SC],
                                    start=(h == 0), stop=(h == G - 1),
                                )
                        nc.scalar.copy(ybuf[:, 0:SC], yps_l[0])
                        nc.vector.tensor_copy(ybuf[:, SC:2 * SC], yps_l[1])
                        nc.gpsimd.dma_start(
                            y[srow:srow + P,
                              grp * 2 * SC:(grp + 1) * 2 * SC], ybuf)

            # software-pipelined emission: attention of chunk sc is emitted
            # before projections of chunk sc+1, so the scheduler prefers
            # the latency-critical attention chain and fills its ACT-bound
            # gaps with projection matmuls. Chunk 0's wo is deferred to the
            # very end, where it fills the exp-bound gaps of the last
            # chunk's attention (which has no projection work left to
            # overlap with).
            proj_rope(0)
            # woT is first needed by wo(1) (~100us in); emitting its DMA
            # here keeps the 12.6us ACT-queue hold behind chunk-0's rope
            # and attention activations
            nc.scalar.dma_start(
                woT_sb, woT.rearrange("(g p) h -> p g h", p=P))
            ots0 = attn(0)
            proj_rope(1)
            ots1 = attn(1)
            proj_rope(2)
            ots2 = attn(2)
            # each wo is emitted one chunk late so its matmuls rank below
            # the next chunk's attention in scheduler priority: they stay
            # in reserve and fill the exp-rate-limited attention windows
            # instead of draining early during the projection passes
            wo_proj(1, ots1)
            proj_rope(3)
            ots3 = attn(NSC - 1)
            wo_proj(2, ots2)
            wo_proj(0, ots0)
            wo_proj(NSC - 1, ots3)

            if dbg:
                nc.sync.dma_start(KR_dbg, KR)
                nc.sync.dma_start(QR_dbg, qr_all)
                nc.sync.dma_start(V_dbg, Vs)

    nc.finalize()
    return nc


def shard_inputs(x, wq, wk, wv, wo, q_norm_w, k_norm_w, cos_table, sin_table,
                 positions, **_ignored):
    """Host-side sharding: returns the list of 8 per-core input maps."""
    x = np.asarray(x, np.float32)
    pos = np.asarray(positions).astype(np.int64)
    cos_sel = np.asarray(cos_table, np.float32)[pos]   # [S, D]
    sin_sel = np.asarray(sin_table, np.float32)[pos]
    qw = np.asarray(q_norm_w, np.float32)
    kw = np.asarray(k_norm_w, np.float32)
    # fold norm weights into the transposed rope tables:
    # w * rope(q') == q'*(w*cos) + rot(q')*(w*sin)
    # fold rotate-half's minus sign into sin rows 0..63:
    # rope(z) = z*cos + [-z2; z1]*sin = z*cos + rot(z)*sin_eff
    # and pre-rotate sin so rot(z)*sin_eff == rot(z * rot(sin_eff))
    sign = np.ones((1, D), np.float32)
    sign[0, :D // 2] = -1.0
    cosq_ = np.ascontiguousarray((cos_sel * qw).T).astype(np.float16)
    sinq_ = np.ascontiguousarray(
        np.roll((sin_sel * qw * sign).T, 64, axis=0)).astype(np.float16)
    cosk_ = np.ascontiguousarray((cos_sel * kw).T).astype(np.float16)
    sink_ = np.ascontiguousarray(
        np.roll((sin_sel * kw * sign).T, 64, axis=0)).astype(np.float16)
    xTf = np.ascontiguousarray(x.reshape(S, HID).T).astype(np.float16)
    wq = np.asarray(wq, np.float32)
    wk = np.asarray(wk, np.float32)
    wv = np.asarray(wv, np.float32)
    wo = np.asarray(wo, np.float32)

    in_maps = []
    for c in range(N_CORES):
        wq_c = wq[c * G * P:(c + 1) * G * P, :].T        # [HID, 512]
        wk_c = wk[c * P:(c + 1) * P, :].T                # [HID, 128]
        wv_c = wv[c * P:(c + 1) * P, :].T                # [HID, 128]
        wqkv = np.concatenate([wq_c, wk_c, wv_c], axis=1)
        m = {
            "xT": xTf,
            "wqkvT": np.ascontiguousarray(wqkv).astype(np.float16),
            "woT": np.ascontiguousarray(
                wo[:, c * G * P:(c + 1) * G * P].T).astype(np.float16),
            "cosq": cosq_, "sinq": sinq_, "cosk": cosk_, "sink": sink_,
        }
        in_maps.append(m)
    return in_maps


_NC = None


def _get_nc():
    global _NC
    if _NC is None:
        _NC = build_program()
    return _NC


def run_on_device(in_maps, trace=False):
    from concourse.bass_utils import run_bass_kernel_spmd
    nc = _get_nc()
    return run_bass_kernel_spmd(nc, in_maps, list(range(N_CORES)), trace=trace)


def kernel(**inputs):
    in_maps = shard_inputs(**inputs)
    res = run_on_device(in_maps).results
    y = np.zeros((S, HID), np.float32)
    for c in range(N_CORES):
        y += res[c]["y"].astype(np.float32)
    return y.reshape(1, S, HID)


# revision 125
# speedup vs baseline: 1.2203x; 1.0001x over previous
"""GQA attention prefill kernel for Trainium2 (Bass/Tile), 8-way tensor
parallel over heads.

Problem (hardcoded): B=1, S=2048, HID=4096, NH=32, KVH=8, D=128, causal
prefill with per-head RMSNorm on q/k and RoPE, positions = arange(S).

Sharding: core c owns kv-head c and q-heads 4c..4c+3. wq/wo sharded on the
head dim, wk/wv on the kv-head dim; x, rope tables replicated. Each core
computes its 4 heads' contribution through wo; the host sums the 8 partial
outputs (partials shipped as fp16, summed in fp32 on host).

Host-side prep (part of sharding): all matmul operands are fed pre-
transposed (contraction dim major) and pre-converted to fp16 — fp16
matmuls run at the same 1 cycle/row as fp32r but halve DMA and SBUF. The
q/k norm weights, the rotate-half sign, and the softmax 1/sqrt(d) are
folded into the rope tables; the sin tables are additionally pre-rotated
by 64 so the on-device rotate-half becomes a single PE matmul against a
fixed permutation matrix (no SBUF->SBUF DMAs).

On-device schedule: one software-pipelined stream per 512-column seq
chunk — projections+rope of chunk sc+1 are emitted after attention of
chunk sc so the scheduler fills the exp-rate-limited attention windows
with projection matmuls (and vice versa fills projection stalls with
attention/wo work). Details:
  - projections: stream x once (quarter-chunk DMAs); q heads in two
    2-head passes and v in per-subtile PSUM tiles to bound PSUM bank
    pressure (a PSUM bank region supports one accumulation group at a
    time). v is produced directly transposed via x-as-stationary
    matmuls.
  - rope+rmsnorm into resident KR / qr tiles; rsqrt is computed as
    exp(-0.5*ln(.)): with a manually preloaded combined {ln,exp,copy,
    square} activation table set the whole kernel needs exactly one
    ACT table load.
  - causal attention computes only the valid rectangle of each
    diagonal k-tile (partial-width PSUM accumulation); the causal mask
    is an additive -30000 upper-triangular constant applied in PSUM by
    a tiny identity-matmul, so nothing sits between the exp and the AV
    matmul. Score exponentials are computed as exp(s - ln16): rmsnorm
    bounds scores at sqrt(128)=11.31, so exp values stay under 5.2e3
    and fit fp16 (the 1/16 cancels between numerator and denominator).
    Denominators accumulate in two alternating fp16 chains on the Pool
    and DVE engines (all-2-byte operands hit the DVE 2x mode); one
    fp16 ones-matmul per (head, chunk) folds the partitions.
  - wo matmul in 2-bank column groups; y partials staged in fp16 and
    written by DMAs issued from the Pool queue so the SP queue never
    delays x prefetches.
PSUM banks are statically split 4 (projections/rope/attention-out/wo)
+ 3 (score tiles, the latency-critical pipeline) + 1 (v subtiles).
"""

import numpy as np

import concourse.bass as bass
import concourse.mybir as mybir
import concourse.tile as tile
from concourse import bacc

P = 128
S = 2048
HID = 4096
D = 128
G = 4            # q heads per core
NHT = HID // P   # 32 h-tiles (contraction)
SC = 512         # seq chunk
NSC = S // SC    # 4
NKT = S // P     # 16 k-tiles
EPS = 1e-6
N_CORES = 8
WQKV = G * P + 2 * P   # 768 packed projection output dims (q|k|v)

F32 = mybir.dt.float32
F16 = mybir.dt.float16
BF16 = mybir.dt.bfloat16

ACT_SET_LN_EXP = 6     # natural_log_exp_and_others: {ln, exp, copy, square}


def build_program(dbg=False):
    nc = bacc.Bacc("TRN2", target_bir_lowering=False, debug=False)
    if dbg:
        KR_dbg = nc.dram_tensor("KR_dbg", [P, S], F16,
                                kind="ExternalOutput").ap()
        QR_dbg = nc.dram_tensor("QR_dbg", [P, G, S], F16,
                                kind="ExternalOutput").ap()
        V_dbg = nc.dram_tensor("V_dbg", [P, NKT, P], BF16,
                               kind="ExternalOutput").ap()
        OT_dbg = nc.dram_tensor("OT_dbg", [P, G, S], F16,
                                kind="ExternalOutput").ap()

    xT = nc.dram_tensor("xT", [HID, S], F16, kind="ExternalInput").ap()
    wqkvT = nc.dram_tensor("wqkvT", [HID, WQKV], F16, kind="ExternalInput").ap()
    woT = nc.dram_tensor("woT", [G * P, HID], F16, kind="ExternalInput").ap()
    cosq = nc.dram_tensor("cosq", [D, S], F16, kind="ExternalInput").ap()
    sinq = nc.dram_tensor("sinq", [D, S], F16, kind="ExternalInput").ap()
    cosk = nc.dram_tensor("cosk", [D, S], F16, kind="ExternalInput").ap()
    sink = nc.dram_tensor("sink", [D, S], F16, kind="ExternalInput").ap()
    y = nc.dram_tensor("y", [S, HID], F16, kind="ExternalOutput").ap()

    Ln = mybir.ActivationFunctionType.Ln
    Exp = mybir.ActivationFunctionType.Exp

    with tile.TileContext(nc) as tc:
        with (
            tc.tile_pool(name="const", bufs=1) as const,
            tc.tile_pool(name="tabs", bufs=2) as tabs,
            tc.tile_pool(name="xw", bufs=4) as xw,
            tc.tile_pool(name="scr", bufs=2) as scr,
            tc.tile_pool(name="rawp", bufs=6) as rawp,
            tc.tile_pool(name="otp", bufs=13) as otp,
            tc.tile_pool(name="ptp", bufs=5) as ptp,
            tc.tile_pool(name="dnp", bufs=7) as dnp,
            tc.tile_pool(name="yp", bufs=2) as yp,
            tc.tile_pool(name="ps", bufs=4, space="PSUM") as ps,
            tc.tile_pool(name="pts", bufs=3, space="PSUM") as pts,
            tc.tile_pool(name="psd", bufs=1, space="PSUM") as psd,
        ):
            # one activation-table load for the whole kernel
            nc.scalar.add_instruction(mybir.InstLoadActFuncSet(
                name=nc.get_next_instruction_name(), ins=[], outs=[],
                act_func_set_id=ACT_SET_LN_EXP))

            # ---- input DMAs, interleaved so x quarters and the weight
            # parts feeding the first projections arrive together; the
            # later weight parts and woT go on the idle ACT queue ----
            xTr = xT.rearrange("(a p) s -> p a s", p=P)
            wqkv_sb = const.tile([P, NHT, WQKV], F16)
            wqkvTr = wqkvT.rearrange("(a p) c -> p a c", p=P)
            xs_first = []
            for qi in range(4):
                xs = xw.tile([P, 8, SC], F16, tag="xs")
                if qi == 0:
                    # first h-tile arrives via the otherwise-idle ACT queue
                    # so the very first projection matmul starts ~1.5us in
                    nc.scalar.dma_start(xs[:, 0:1, :], xTr[:, 0:1, 0:SC])
                    nc.scalar.dma_start(wqkv_sb[:, 0:1, :],
                                        wqkvTr[:, 0:1, :])
                    nc.sync.dma_start(xs[:, 1:8, :], xTr[:, 1:8, 0:SC])
                    nc.sync.dma_start(wqkv_sb[:, 1:4, :], wqkvTr[:, 1:4, :])
                else:
                    nc.sync.dma_start(xs, xTr[:, 8 * qi:8 * (qi + 1), 0:SC])
                    nc.sync.dma_start(wqkv_sb[:, 8 * qi:8 * qi + 4, :],
                                      wqkvTr[:, 8 * qi:8 * qi + 4, :])
                xs_first.append(xs)
                nc.scalar.dma_start(wqkv_sb[:, 8 * qi + 4:8 * qi + 8, :],
                                    wqkvTr[:, 8 * qi + 4:8 * qi + 8, :])
            woT_sb = const.tile([P, G, HID], F16)

            # ---- constants ----------------------------------------------
            f32tmp = const.tile([P, SC], F32)
            f32tmp2 = const.tile([P, P], F32)

            # ones[k, m] == 1: matmul(out, ones, rhs) -> column sums of rhs
            # broadcast across all 128 output partitions.
            ones16 = const.tile([P, P], F16)
            nc.gpsimd.memset(f32tmp, 1.0)
            nc.vector.tensor_copy(ones16, f32tmp[:, 0:P])

            # rotP: lhsT permutation with rotP[k, i] = 1 iff i == (k+64)%128,
            # so matmul(out, rotP, v) = v rotated-half along partitions.
            nc.gpsimd.memset(f32tmp2, 1.0)
            rsel1 = const.tile([P, P], F32)
            nc.gpsimd.affine_select(
                rsel1, f32tmp2, pattern=[[1, P]],
                compare_op=mybir.AluOpType.is_equal,
                fill=0.0, base=-64, channel_multiplier=-1,
            )
            rsel2 = const.tile([P, P], F32)
            nc.gpsimd.affine_select(
                rsel2, f32tmp2, pattern=[[1, P]],
                compare_op=mybir.AluOpType.is_equal,
                fill=0.0, base=64, channel_multiplier=-1,
            )
            nc.vector.tensor_add(rsel1, rsel1, rsel2)
            rotP = const.tile([P, P], F16)
            nc.vector.tensor_copy(rotP, rsel1)

            # additive causal mask for the exact-diagonal 128x128 tiles:
            # -30000 where q_local < k_local (exp -> 0), else 0. Applied in
            # PSUM via matmul(identity, umask) accumulation so no vector op
            # sits between the exp and the AV matmul.
            ident16 = const.tile([P, P], F16)
            nc.gpsimd.memset(f32tmp2, 1.0)
            isel = const.tile([P, P], F32)
            nc.gpsimd.affine_select(
                isel, f32tmp2, pattern=[[1, P]],
                compare_op=mybir.AluOpType.is_equal,
                fill=0.0, base=0, channel_multiplier=-1,
            )
            nc.vector.tensor_copy(ident16, isel)
            nc.gpsimd.memset(f32tmp2, -30000.0)
            usel = const.tile([P, P], F32)
            nc.gpsimd.affine_select(
                usel, f32tmp2, pattern=[[-1, P]],
                compare_op=mybir.AluOpType.is_ge,
                fill=0.0, base=-1, channel_multiplier=1,
            )
            umask16 = const.tile([P, P], F16)
            nc.vector.tensor_copy(umask16, usel)

            bias_keps = const.tile([P, 1], F32)
            nc.gpsimd.memset(bias_keps, float(P) * EPS)
            bias_qeps = const.tile([P, 1], F32)
            nc.gpsimd.memset(bias_qeps, EPS)
            # exp(s - ln16): keeps the fp16 softmax-denominator
            # accumulators in range; cancels between numerator/denominator
            bias_ln16 = const.tile([P, 1], F32)
            nc.gpsimd.memset(bias_ln16, -2.7725887)

            # ---- resident tensors ---------------------------------------
            KR = const.tile([P, S], F16)          # roped+scaled K, [d, s]
            Vs = const.tile([P, NKT, P], BF16)    # V, [s-in-tile, k-tile, d]
            qr_all = const.tile([P, G, S], F16)   # roped+scaled Q, [d, h, s]

            # ============ per-chunk emitters ============================
            def rope_one(zraw, cos_t, sin_t, out_ap, bias_ap, ln_scale):
                sq = scr.tile([P, SC], F16, tag="sq")
                nc.vector.tensor_mul(sq, zraw, zraw)
                ssb = pts.tile([P, SC], F32, tag="pts")
                nc.tensor.matmul(ssb, ones16, sq, start=True, stop=True)
                # 1/sqrt(ssq + d*eps) == exp(-0.5*ln(ssq + d*eps)):
                # the rms-norm (and for K the softmax 1/sqrt(d)) in one go
                ln_ = scr.tile([P, SC], F32, tag="lnk")
                nc.scalar.activation(ln_, ssb, Ln, bias=bias_ap,
                                     scale=ln_scale)
                rf = scr.tile([P, SC], F32, tag="rk")
                nc.scalar.activation(rf, ln_, Exp, bias=0.0, scale=-0.5)
                # rope: out = z*cos + rot(z*sin_pre), sin pre-rotated on
                # host; the two SBUF-only muls run on the idle Pool engine
                # so the DVE chain (which must touch PSUM) stays short
                t1 = scr.tile([P, SC], F16, tag="t1")
                nc.gpsimd.tensor_mul(t1, zraw, sin_t)
                rps = pts.tile([P, SC], F32, tag="pts")
                nc.tensor.matmul(rps, rotP, t1, start=True, stop=True)
                pre = scr.tile([P, SC], F32, tag="kpre")
                nc.gpsimd.tensor_mul(pre, zraw, cos_t)
                nc.vector.tensor_add(pre, pre, rps)
                nc.vector.tensor_mul(out_ap, pre, rf)

            def proj_rope(sc):
                q0 = sc * SC
                # x streamed in 4 quarter-chunk tiles of 8 h-tiles each;
                # q projection runs in two passes of 2 heads to keep at
                # most 3 projection PSUM banks live at a time
                xss = []
                qps = [None] * G
                qps[0] = ps.tile([P, SC], F32, tag="ps", name="qps0")
                qps[1] = ps.tile([P, SC], F32, tag="ps", name="qps1")
                kps = ps.tile([P, SC], F32, tag="ps")
                for qi in range(4):
                    if sc == 0:
                        xs = xs_first[qi]
                    else:
                        xs = xw.tile([P, 8, SC], F16, tag="xs")
                        nc.sync.dma_start(
                            xs, xTr[:, 8 * qi:8 * (qi + 1), q0:q0 + SC])
                    xss.append(xs)
                    for ht8 in range(8):
                        ht = qi * 8 + ht8
                        xt = xs[:, ht8, :]
                        st = ht == 0
                        sp = ht == NHT - 1
                        for mt in range(2):
                            nc.tensor.matmul(
                                qps[mt],
                                wqkv_sb[:, ht, mt * P:(mt + 1) * P], xt,
                                start=st, stop=sp,
                            )
                        nc.tensor.matmul(
                            kps, wqkv_sb[:, ht, G * P:G * P + P], xt,
                            start=st, stop=sp,
                        )

                cq = tabs.tile([P, SC], F16, tag="cosq")
                nc.sync.dma_start(cq, cosq[:, q0:q0 + SC])
                sq_ = tabs.tile([P, SC], F16, tag="sinq")
                nc.sync.dma_start(sq_, sinq[:, q0:q0 + SC])
                ck = tabs.tile([P, SC], F16, tag="cosk")
                nc.sync.dma_start(ck, cosk[:, q0:q0 + SC])
                sk = tabs.tile([P, SC], F16, tag="sink")
                nc.sync.dma_start(sk, sink[:, q0:q0 + SC])

                # PSUM-freeing copies first; for sc>0 the q0/q1 ropes run
                # before the K rope so this chunk's attention (whose
                # off-diagonal tiles only need earlier chunks' KR) can
                # start as early as possible — the K rope only gates the
                # diagonal tiles, which come last in the k-tile order
                kraw = rawp.tile([P, SC], F16, tag="raw")
                nc.scalar.copy(kraw, kps)
                qraws = [None] * G
                for h in range(2):
                    qraw = rawp.tile([P, SC], F16, tag="raw")
                    nc.scalar.copy(qraw, qps[h])
                    qraws[h] = qraw
                if sc == 0:
                    rope_one(kraw, ck, sk, KR[:, q0:q0 + SC],
                             bias_keps, 1.0)
                else:
                    rope_one(qraws[0], cq, sq_,
                             qr_all[:, 0, q0:q0 + SC], bias_qeps, 1.0 / P)
                    rope_one(qraws[1], cq, sq_,
                             qr_all[:, 1, q0:q0 + SC], bias_qeps, 1.0 / P)

                # pass B: q heads 2,3
                qps[2] = ps.tile([P, SC], F32, tag="ps", name="qps2")
                qps[3] = ps.tile([P, SC], F32, tag="ps", name="qps3")
                for ht in range(NHT):
                    xt = xss[ht // 8][:, ht % 8, :]
                    st = ht == 0
                    sp = ht == NHT - 1
                    for mt in range(2, G):
                        nc.tensor.matmul(
                            qps[mt], wqkv_sb[:, ht, mt * P:(mt + 1) * P],
                            xt, start=st, stop=sp,
                        )

                # v, directly transposed: out[s, d]; each 128-row s-subtile
                # accumulates in its own small PSUM tile (a PSUM bank region
                # only supports one accumulation group at a time)
                for j in range(4):
                    vpsd = psd.tile([P, P], F32, tag="psq")
                    for ht in range(NHT):
                        nc.tensor.matmul(
                            vpsd,
                            xss[ht // 8][:, ht % 8, j * P:(j + 1) * P],
                            wqkv_sb[:, ht, G * P + P:],
                            start=(ht == 0), stop=(ht == NHT - 1),
                        )
                    nc.scalar.copy(Vs[:, sc * 4 + j, :], vpsd)

                for h in range(2, G):
                    qraw = rawp.tile([P, SC], F16, tag="raw")
                    nc.scalar.copy(qraw, qps[h])
                    qraws[h] = qraw
                if sc == 0:
                    for h in range(2):
                        rope_one(qraws[h], cq, sq_,
                                 qr_all[:, h, q0:q0 + SC], bias_qeps, 1.0 / P)
                for h in range(2, G):
                    rope_one(qraws[h], cq, sq_,
                             qr_all[:, h, q0:q0 + SC], bias_qeps, 1.0 / P)
                if sc > 0:
                    rope_one(kraw, ck, sk, KR[:, q0:q0 + SC],
                             bias_keps, 1.0)

            def attn(sc):
                q0 = sc * SC
                ots = []
                off = sc * 4          # full (off-diagonal) k-tiles
                nkt = off + 4
                for h in range(G):
                    qrh = qr_all[:, h, q0:q0 + SC]
                    avp = ps.tile([P, SC], F32, tag="ps")
                    # two denominator accumulators: even k-tiles on Pool,
                    # odd on DVE — two independent serial chains. exp is
                    # computed as exp(s - ln16), which bounds the fp16
                    # accumulators (the 1/16 cancels between numerator and
                    # denominator); all-2-byte operands let the DVE chain
                    # run in its 2x mode
                    dnacc = [dnp.tile([P, SC], F16, tag="dn", name=f"dn{i}")
                             for i in range(2)]
                    if off == 0:
                        # first chunk: diagonal rectangles don't cover the
                        # accumulators' full width, so zero-init them
                        nc.gpsimd.memset(dnacc[0], 0.0)
                        nc.vector.memset(dnacc[1], 0.0)
                    for kt in range(nkt):
                        # diagonal k-tile kt2 only attends q columns
                        # >= 128*kt2: compute the valid rectangle only
                        c0 = (kt - off) * P if kt >= off else 0
                        ptps = pts.tile([P, SC], F32, tag="pts")
                        diag = kt >= off
                        nc.tensor.matmul(
                            ptps[:, c0:], KR[:, kt * P:(kt + 1) * P],
                            qrh[:, c0:],
                            start=True, stop=not diag,
                        )
                        if diag:
                            # additive causal mask for the leading 128 q
                            # columns (the exact-diagonal subtile)
                            nc.tensor.matmul(
                                ptps[:, c0:c0 + P], ident16, umask16,
                                start=False, stop=True,
                            )
                        pt = ptp.tile([P, SC], BF16, tag="pt")
                        nc.scalar.activation(pt[:, c0:], ptps[:, c0:],
                                             Exp, bias=bias_ln16, scale=1.0)
                        eng = nc.gpsimd if kt % 2 == 0 else nc.vector
                        acc = dnacc[kt % 2]
                        if kt < 2 and off > 0:
                            eng.tensor_copy(acc, pt)
                        else:
                            eng.tensor_add(acc[:, c0:], acc[:, c0:],
                                           pt[:, c0:])
                        nc.tensor.matmul(avp[:, c0:], Vs[:, kt, :],
                                         pt[:, c0:],
                                         start=(kt == 0),
                                         stop=(kt == nkt - 1))
                    # fold the 128 partition-partials of the denominator
                    dnps = ps.tile([P, SC], F32, tag="ps")
                    for i, acc in enumerate(dnacc):
                        nc.tensor.matmul(dnps, ones16, acc,
                                         start=(i == 0),
                                         stop=(i == len(dnacc) - 1))
                    rcp = scr.tile([P, SC], F32, tag="rcp")
                    nc.vector.reciprocal(rcp, dnps)
                    ot = otp.tile([P, SC], F16, tag="ot")
                    nc.vector.tensor_mul(ot, avp, rcp)
                    ots.append(ot)
                    if dbg:
                        nc.gpsimd.dma_start(OT_dbg[:, h, q0:q0 + SC], ot)
                return ots

            def wo_proj(sc, ots):
                q0 = sc * SC
                # output projection: 4 column groups of 2 PSUM banks
                for stl in range(SC // P):
                    srow = q0 + stl * P
                    for grp in range(4):
                        ybuf = yp.tile([P, 2 * SC], F16, tag="ys")
                        yps_l = [ps.tile([P, SC], F32, tag="ps",
                                         name=f"yps{j}")
                                 for j in range(2)]
                        for h in range(G):
                            lhs = ots[h][:, stl * P:(stl + 1) * P]
                            for j in range(2):
                                hc = grp * 2 + j
                                nc.tensor.matmul(
                                    yps_l[j], lhs,
                                    woT_sb[:, h, hc * SC:(hc + 1) * SC],
                                    start=(h == 0), stop=(h == G - 1),
                                )
                        nc.scalar.copy(ybuf[:, 0:SC], yps_l[0])
                        nc.vector.tensor_copy(ybuf[:, SC:2 * SC], yps_l[1])
                        c0 = grp * 2 * SC
                        if sc == NSC - 1 and stl == SC // P - 1 and grp == 3:
                            # last group: ship each half as its copy lands
                            # so the kernel-tail drain isn't gated by the
                            # full copy+copy+DMA chain
                            nc.gpsimd.dma_start(
                                y[srow:srow + P, c0:c0 + SC], ybuf[:, 0:SC])
                            nc.gpsimd.dma_start(
                                y[srow:srow + P, c0 + SC:c0 + 2 * SC],
                                ybuf[:, SC:2 * SC])
                        else:
                            nc.gpsimd.dma_start(
                                y[srow:srow + P, c0:c0 + 2 * SC], ybuf)

            # software-pipelined emission: attention of chunk sc is emitted
            # before projections of chunk sc+1, so the scheduler prefers
            # the latency-critical attention chain and fills its ACT-bound
            # gaps with projection matmuls. Chunk 0's wo is deferred to the
            # very end, where it fills the exp-bound gaps of the last
            # chunk's attention (which has no projection work left to
            # overlap with).
            proj_rope(0)
            # woT is first needed by wo(1) (~100us in); emitting its DMA
            # here keeps the 12.6us ACT-queue hold behind chunk-0's rope
            # and attention activations
            nc.scalar.dma_start(
                woT_sb, woT.rearrange("(g p) h -> p g h", p=P))
            ots0 = attn(0)
            proj_rope(1)
            ots1 = attn(1)
            proj_rope(2)
            ots2 = attn(2)
            # each wo is emitted one chunk late so its matmuls rank below
            # the next chunk's attention in scheduler priority: they stay
            # in reserve and fill the exp-rate-limited attention windows
            # instead of draining early during the projection passes
            wo_proj(1, ots1)
            proj_rope(3)
            ots3 = attn(NSC - 1)
            wo_proj(2, ots2)
            wo_proj(0, ots0)
            wo_proj(NSC - 1, ots3)

            if dbg:
                nc.sync.dma_start(KR_dbg, KR)
                nc.sync.dma_start(QR_dbg, qr_all)
                nc.sync.dma_start(V_dbg, Vs)

    nc.finalize()
    return nc


def shard_inputs(x, wq, wk, wv, wo, q_norm_w, k_norm_w, cos_table, sin_table,
                 positions, **_ignored):
    """Host-side sharding: returns the list of 8 per-core input maps."""
    x = np.asarray(x, np.float32)
    pos = np.asarray(positions).astype(np.int64)
    cos_sel = np.asarray(cos_table, np.float32)[pos]   # [S, D]
    sin_sel = np.asarray(sin_table, np.float32)[pos]
    qw = np.asarray(q_norm_w, np.float32)
    kw = np.asarray(k_norm_w, np.float32)
    # fold norm weights into the transposed rope tables:
    # w * rope(q') == q'*(w*cos) + rot(q')*(w*sin)
    # fold rotate-half's minus sign into sin rows 0..63:
    # rope(z) = z*cos + [-z2; z1]*sin = z*cos + rot(z)*sin_eff
    # and pre-rotate sin so rot(z)*sin_eff == rot(z * rot(sin_eff))
    sign = np.ones((1, D), np.float32)
    sign[0, :D // 2] = -1.0
    cosq_ = np.ascontiguousarray((cos_sel * qw).T).astype(np.float16)
    sinq_ = np.ascontiguousarray(
        np.roll((sin_sel * qw * sign).T, 64, axis=0)).astype(np.float16)
    cosk_ = np.ascontiguousarray((cos_sel * kw).T).astype(np.float16)
    sink_ = np.ascontiguousarray(
        np.roll((sin_sel * kw * sign).T, 64, axis=0)).astype(np.float16)
    xTf = np.ascontiguousarray(x.reshape(S, HID).T).astype(np.float16)
    wq = np.asarray(wq, np.float32)
    wk = np.asarray(wk, np.float32)
    wv = np.asarray(wv, np.float32)
    wo = np.asarray(wo, np.float32)

    in_maps = []
    for c in range(N_CORES):
        wq_c = wq[c * G * P:(c + 1) * G * P, :].T        # [HID, 512]
        wk_c = wk[c * P:(c + 1) * P, :].T                # [HID, 128]
        wv_c = wv[c * P:(c + 1) * P, :].T                # [HID, 128]
        wqkv = np.concatenate([wq_c, wk_c, wv_c], axis=1)
        m = {
            "xT": xTf,
            "wqkvT": np.ascontiguousarray(wqkv).astype(np.float16),
            "woT": np.ascontiguousarray(
                wo[:, c * G * P:(c + 1) * G * P].T).astype(np.float16),
            "cosq": cosq_, "sinq": sinq_, "cosk": cosk_, "sink": sink_,
        }
        in_maps.append(m)
    return in_maps


_NC = None


def _get_nc():
    global _NC
    if _NC is None:
        _NC = build_program()
    return _NC


def run_on_device(in_maps, trace=False):
    from concourse.bass_utils import run_bass_kernel_spmd
    nc = _get_nc()
    return run_bass_kernel_spmd(nc, in_maps, list(range(N_CORES)), trace=trace)


def kernel(**inputs):
    in_maps = shard_inputs(**inputs)
    res = run_on_device(in_maps).results
    y = np.zeros((S, HID), np.float32)
    for c in range(N_CORES):
        y += res[c]["y"].astype(np.float32)
    return y.reshape(1, S, HID)


# revision 126
# speedup vs baseline: 1.2213x; 1.0008x over previous
"""GQA attention prefill kernel for Trainium2 (Bass/Tile), 8-way tensor
parallel over heads.

Problem (hardcoded): B=1, S=2048, HID=4096, NH=32, KVH=8, D=128, causal
prefill with per-head RMSNorm on q/k and RoPE, positions = arange(S).

Sharding: core c owns kv-head c and q-heads 4c..4c+3. wq/wo sharded on the
head dim, wk/wv on the kv-head dim; x, rope tables replicated. Each core
computes its 4 heads' contribution through wo; the host sums the 8 partial
outputs (partials shipped as fp16, summed in fp32 on host).

Host-side prep (part of sharding): all matmul operands are fed pre-
transposed (contraction dim major) and pre-converted to fp16 — fp16
matmuls run at the same 1 cycle/row as fp32r but halve DMA and SBUF. The
q/k norm weights, the rotate-half sign, and the softmax 1/sqrt(d) are
folded into the rope tables; the sin tables are additionally pre-rotated
by 64 so the on-device rotate-half becomes a single PE matmul against a
fixed permutation matrix (no SBUF->SBUF DMAs).

On-device schedule: one software-pipelined stream per 512-column seq
chunk — projections+rope of chunk sc+1 are emitted after attention of
chunk sc so the scheduler fills the exp-rate-limited attention windows
with projection matmuls (and vice versa fills projection stalls with
attention/wo work). Details:
  - projections: stream x once (quarter-chunk DMAs); q heads in two
    2-head passes and v in per-subtile PSUM tiles to bound PSUM bank
    pressure (a PSUM bank region supports one accumulation group at a
    time). v is produced directly transposed via x-as-stationary
    matmuls.
  - rope+rmsnorm into resident KR / qr tiles; rsqrt is computed as
    exp(-0.5*ln(.)): with a manually preloaded combined {ln,exp,copy,
    square} activation table set the whole kernel needs exactly one
    ACT table load.
  - causal attention computes only the valid rectangle of each
    diagonal k-tile (partial-width PSUM accumulation); the causal mask
    is an additive -30000 upper-triangular constant applied in PSUM by
    a tiny identity-matmul, so nothing sits between the exp and the AV
    matmul. Score exponentials are computed as exp(s - ln16): rmsnorm
    bounds scores at sqrt(128)=11.31, so exp values stay under 5.2e3
    and fit fp16 (the 1/16 cancels between numerator and denominator).
    Denominators accumulate in two alternating fp16 chains on the Pool
    and DVE engines (all-2-byte operands hit the DVE 2x mode); one
    fp16 ones-matmul per (head, chunk) folds the partitions.
  - wo matmul in 2-bank column groups; y partials staged in fp16 and
    written by DMAs issued from the Pool queue so the SP queue never
    delays x prefetches.
PSUM banks are statically split 4 (projections/rope/attention-out/wo)
+ 3 (score tiles, the latency-critical pipeline) + 1 (v subtiles).
"""

import numpy as np

import concourse.bass as bass
import concourse.mybir as mybir
import concourse.tile as tile
from concourse import bacc

P = 128
S = 2048
HID = 4096
D = 128
G = 4            # q heads per core
NHT = HID // P   # 32 h-tiles (contraction)
SC = 512         # seq chunk
NSC = S // SC    # 4
NKT = S // P     # 16 k-tiles
EPS = 1e-6
N_CORES = 8
WQKV = G * P + 2 * P   # 768 packed projection output dims (q|k|v)

F32 = mybir.dt.float32
F16 = mybir.dt.float16
BF16 = mybir.dt.bfloat16

ACT_SET_LN_EXP = 6     # natural_log_exp_and_others: {ln, exp, copy, square}


def build_program(dbg=False):
    nc = bacc.Bacc("TRN2", target_bir_lowering=False, debug=False)
    if dbg:
        KR_dbg = nc.dram_tensor("KR_dbg", [P, S], F16,
                                kind="ExternalOutput").ap()
        QR_dbg = nc.dram_tensor("QR_dbg", [P, G, S], F16,
                                kind="ExternalOutput").ap()
        V_dbg = nc.dram_tensor("V_dbg", [P, NKT, P], BF16,
                               kind="ExternalOutput").ap()
        OT_dbg = nc.dram_tensor("OT_dbg", [P, G, S], F16,
                                kind="ExternalOutput").ap()

    xT = nc.dram_tensor("xT", [HID, S], F16, kind="ExternalInput").ap()
    wqkvT = nc.dram_tensor("wqkvT", [HID, WQKV], F16, kind="ExternalInput").ap()
    woT = nc.dram_tensor("woT", [G * P, HID], F16, kind="ExternalInput").ap()
    cosq = nc.dram_tensor("cosq", [D, S], F16, kind="ExternalInput").ap()
    sinq = nc.dram_tensor("sinq", [D, S], F16, kind="ExternalInput").ap()
    cosk = nc.dram_tensor("cosk", [D, S], F16, kind="ExternalInput").ap()
    sink = nc.dram_tensor("sink", [D, S], F16, kind="ExternalInput").ap()
    y = nc.dram_tensor("y", [S, HID], F16, kind="ExternalOutput").ap()

    Ln = mybir.ActivationFunctionType.Ln
    Exp = mybir.ActivationFunctionType.Exp

    with tile.TileContext(nc) as tc:
        with (
            tc.tile_pool(name="const", bufs=1) as const,
            tc.tile_pool(name="tabs", bufs=2) as tabs,
            tc.tile_pool(name="xw", bufs=4) as xw,
            tc.tile_pool(name="scr", bufs=2) as scr,
            tc.tile_pool(name="rawp", bufs=6) as rawp,
            tc.tile_pool(name="otp", bufs=13) as otp,
            tc.tile_pool(name="ptp", bufs=5) as ptp,
            tc.tile_pool(name="dnp", bufs=7) as dnp,
            tc.tile_pool(name="yp", bufs=2) as yp,
            tc.tile_pool(name="ps", bufs=4, space="PSUM") as ps,
            tc.tile_pool(name="pts", bufs=3, space="PSUM") as pts,
            tc.tile_pool(name="psd", bufs=1, space="PSUM") as psd,
        ):
            # one activation-table load for the whole kernel
            nc.scalar.add_instruction(mybir.InstLoadActFuncSet(
                name=nc.get_next_instruction_name(), ins=[], outs=[],
                act_func_set_id=ACT_SET_LN_EXP))

            # ---- input DMAs, interleaved so x quarters and the weight
            # parts feeding the first projections arrive together; the
            # later weight parts and woT go on the idle ACT queue ----
            xTr = xT.rearrange("(a p) s -> p a s", p=P)
            wqkv_sb = const.tile([P, NHT, WQKV], F16)
            wqkvTr = wqkvT.rearrange("(a p) c -> p a c", p=P)
            xs_first = []
            for qi in range(4):
                xs = xw.tile([P, 8, SC], F16, tag="xs")
                if qi == 0:
                    # first h-tile arrives via the otherwise-idle ACT queue
                    # so the very first projection matmul starts ~1.5us in
                    nc.scalar.dma_start(xs[:, 0:1, :], xTr[:, 0:1, 0:SC])
                    nc.scalar.dma_start(wqkv_sb[:, 0:1, :],
                                        wqkvTr[:, 0:1, :])
                    nc.sync.dma_start(xs[:, 1:8, :], xTr[:, 1:8, 0:SC])
                    nc.sync.dma_start(wqkv_sb[:, 1:4, :], wqkvTr[:, 1:4, :])
                else:
                    nc.sync.dma_start(xs, xTr[:, 8 * qi:8 * (qi + 1), 0:SC])
                    nc.sync.dma_start(wqkv_sb[:, 8 * qi:8 * qi + 4, :],
                                      wqkvTr[:, 8 * qi:8 * qi + 4, :])
                xs_first.append(xs)
                nc.scalar.dma_start(wqkv_sb[:, 8 * qi + 4:8 * qi + 8, :],
                                    wqkvTr[:, 8 * qi + 4:8 * qi + 8, :])
            woT_sb = const.tile([P, G, HID], F16)

            # ---- constants ----------------------------------------------
            f32tmp = const.tile([P, SC], F32)
            f32tmp2 = const.tile([P, P], F32)

            # ones[k, m] == 1: matmul(out, ones, rhs) -> column sums of rhs
            # broadcast across all 128 output partitions.
            ones16 = const.tile([P, P], F16)
            nc.gpsimd.memset(f32tmp, 1.0)
            nc.vector.tensor_copy(ones16, f32tmp[:, 0:P])

            # rotP: lhsT permutation with rotP[k, i] = 1 iff i == (k+64)%128,
            # so matmul(out, rotP, v) = v rotated-half along partitions.
            nc.gpsimd.memset(f32tmp2, 1.0)
            rsel1 = const.tile([P, P], F32)
            nc.gpsimd.affine_select(
                rsel1, f32tmp2, pattern=[[1, P]],
                compare_op=mybir.AluOpType.is_equal,
                fill=0.0, base=-64, channel_multiplier=-1,
            )
            rsel2 = const.tile([P, P], F32)
            nc.gpsimd.affine_select(
                rsel2, f32tmp2, pattern=[[1, P]],
                compare_op=mybir.AluOpType.is_equal,
                fill=0.0, base=64, channel_multiplier=-1,
            )
            nc.vector.tensor_add(rsel1, rsel1, rsel2)
            rotP = const.tile([P, P], F16)
            nc.vector.tensor_copy(rotP, rsel1)

            # additive causal mask for the exact-diagonal 128x128 tiles:
            # -30000 where q_local < k_local (exp -> 0), else 0. Applied in
            # PSUM via matmul(identity, umask) accumulation so no vector op
            # sits between the exp and the AV matmul.
            ident16 = const.tile([P, P], F16)
            nc.gpsimd.memset(f32tmp2, 1.0)
            isel = const.tile([P, P], F32)
            nc.gpsimd.affine_select(
                isel, f32tmp2, pattern=[[1, P]],
                compare_op=mybir.AluOpType.is_equal,
                fill=0.0, base=0, channel_multiplier=-1,
            )
            nc.vector.tensor_copy(ident16, isel)
            nc.gpsimd.memset(f32tmp2, -30000.0)
            usel = const.tile([P, P], F32)
            nc.gpsimd.affine_select(
                usel, f32tmp2, pattern=[[-1, P]],
                compare_op=mybir.AluOpType.is_ge,
                fill=0.0, base=-1, channel_multiplier=1,
            )
            umask16 = const.tile([P, P], F16)
            nc.vector.tensor_copy(umask16, usel)

            bias_keps = const.tile([P, 1], F32)
            nc.gpsimd.memset(bias_keps, float(P) * EPS)
            bias_qeps = const.tile([P, 1], F32)
            nc.gpsimd.memset(bias_qeps, EPS)
            # exp(s - ln16): keeps the fp16 softmax-denominator
            # accumulators in range; cancels between numerator/denominator
            bias_ln16 = const.tile([P, 1], F32)
            nc.gpsimd.memset(bias_ln16, -2.7725887)

            # ---- resident tensors ---------------------------------------
            KR = const.tile([P, S], F16)          # roped+scaled K, [d, s]
            Vs = const.tile([P, NKT, P], BF16)    # V, [s-in-tile, k-tile, d]
            qr_all = const.tile([P, G, S], F16)   # roped+scaled Q, [d, h, s]

            # ============ per-chunk emitters ============================
            def rope_one(zraw, cos_t, sin_t, out_ap, bias_ap, ln_scale):
                sq = scr.tile([P, SC], F16, tag="sq")
                nc.vector.tensor_mul(sq, zraw, zraw)
                ssb = pts.tile([P, SC], F32, tag="pts")
                nc.tensor.matmul(ssb, ones16, sq, start=True, stop=True)
                # 1/sqrt(ssq + d*eps) == exp(-0.5*ln(ssq + d*eps)):
                # the rms-norm (and for K the softmax 1/sqrt(d)) in one go
                ln_ = scr.tile([P, SC], F32, tag="lnk")
                nc.scalar.activation(ln_, ssb, Ln, bias=bias_ap,
                                     scale=ln_scale)
                rf = scr.tile([P, SC], F32, tag="rk")
                nc.scalar.activation(rf, ln_, Exp, bias=0.0, scale=-0.5)
                # rope: out = z*cos + rot(z*sin_pre), sin pre-rotated on
                # host; the two SBUF-only muls run on the idle Pool engine
                # so the DVE chain (which must touch PSUM) stays short
                t1 = scr.tile([P, SC], F16, tag="t1")
                nc.gpsimd.tensor_mul(t1, zraw, sin_t)
                rps = pts.tile([P, SC], F32, tag="pts")
                nc.tensor.matmul(rps, rotP, t1, start=True, stop=True)
                pre = scr.tile([P, SC], F32, tag="kpre")
                nc.gpsimd.tensor_mul(pre, zraw, cos_t)
                nc.vector.tensor_add(pre, pre, rps)
                nc.vector.tensor_mul(out_ap, pre, rf)

            def proj_rope(sc):
                q0 = sc * SC
                # x streamed in 4 quarter-chunk tiles of 8 h-tiles each;
                # q projection runs in two passes of 2 heads to keep at
                # most 3 projection PSUM banks live at a time
                xss = []
                qps = [None] * G
                qps[0] = ps.tile([P, SC], F32, tag="ps", name="qps0")
                qps[1] = ps.tile([P, SC], F32, tag="ps", name="qps1")
                kps = ps.tile([P, SC], F32, tag="ps")
                for qi in range(4):
                    if sc == 0:
                        xs = xs_first[qi]
                    else:
                        xs = xw.tile([P, 8, SC], F16, tag="xs")
                        nc.sync.dma_start(
                            xs, xTr[:, 8 * qi:8 * (qi + 1), q0:q0 + SC])
                    xss.append(xs)
                    for ht8 in range(8):
                        ht = qi * 8 + ht8
                        xt = xs[:, ht8, :]
                        st = ht == 0
                        sp = ht == NHT - 1
                        for mt in range(2):
                            nc.tensor.matmul(
                                qps[mt],
                                wqkv_sb[:, ht, mt * P:(mt + 1) * P], xt,
                                start=st, stop=sp,
                            )
                        nc.tensor.matmul(
                            kps, wqkv_sb[:, ht, G * P:G * P + P], xt,
                            start=st, stop=sp,
                        )

                cq = tabs.tile([P, SC], F16, tag="cosq")
                nc.sync.dma_start(cq, cosq[:, q0:q0 + SC])
                sq_ = tabs.tile([P, SC], F16, tag="sinq")
                nc.sync.dma_start(sq_, sinq[:, q0:q0 + SC])
                ck = tabs.tile([P, SC], F16, tag="cosk")
                nc.sync.dma_start(ck, cosk[:, q0:q0 + SC])
                sk = tabs.tile([P, SC], F16, tag="sink")
                nc.sync.dma_start(sk, sink[:, q0:q0 + SC])

                # PSUM-freeing copies first; for sc>0 the q0/q1 ropes run
                # before the K rope so this chunk's attention (whose
                # off-diagonal tiles only need earlier chunks' KR) can
                # start as early as possible — the K rope only gates the
                # diagonal tiles, which come last in the k-tile order
                kraw = rawp.tile([P, SC], F16, tag="raw")
                nc.scalar.copy(kraw, kps)
                qraws = [None] * G
                for h in range(2):
                    qraw = rawp.tile([P, SC], F16, tag="raw")
                    nc.scalar.copy(qraw, qps[h])
                    qraws[h] = qraw
                if sc == 0:
                    rope_one(kraw, ck, sk, KR[:, q0:q0 + SC],
                             bias_keps, 1.0)
                else:
                    rope_one(qraws[0], cq, sq_,
                             qr_all[:, 0, q0:q0 + SC], bias_qeps, 1.0 / P)
                    rope_one(qraws[1], cq, sq_,
                             qr_all[:, 1, q0:q0 + SC], bias_qeps, 1.0 / P)

                # pass B: q heads 2,3
                qps[2] = ps.tile([P, SC], F32, tag="ps", name="qps2")
                qps[3] = ps.tile([P, SC], F32, tag="ps", name="qps3")
                for ht in range(NHT):
                    xt = xss[ht // 8][:, ht % 8, :]
                    st = ht == 0
                    sp = ht == NHT - 1
                    for mt in range(2, G):
                        nc.tensor.matmul(
                            qps[mt], wqkv_sb[:, ht, mt * P:(mt + 1) * P],
                            xt, start=st, stop=sp,
                        )

                # v, directly transposed: out[s, d]; each 128-row s-subtile
                # accumulates in its own small PSUM tile (a PSUM bank region
                # only supports one accumulation group at a time)
                for j in range(4):
                    vpsd = psd.tile([P, P], F32, tag="psq")
                    for ht in range(NHT):
                        nc.tensor.matmul(
                            vpsd,
                            xss[ht // 8][:, ht % 8, j * P:(j + 1) * P],
                            wqkv_sb[:, ht, G * P + P:],
                            start=(ht == 0), stop=(ht == NHT - 1),
                        )
                    nc.scalar.copy(Vs[:, sc * 4 + j, :], vpsd)

                for h in range(2, G):
                    qraw = rawp.tile([P, SC], F16, tag="raw")
                    nc.scalar.copy(qraw, qps[h])
                    qraws[h] = qraw
                if sc == 0:
                    for h in range(2):
                        rope_one(qraws[h], cq, sq_,
                                 qr_all[:, h, q0:q0 + SC], bias_qeps, 1.0 / P)
                for h in range(2, G):
                    rope_one(qraws[h], cq, sq_,
                             qr_all[:, h, q0:q0 + SC], bias_qeps, 1.0 / P)
                if sc > 0:
                    rope_one(kraw, ck, sk, KR[:, q0:q0 + SC],
                             bias_keps, 1.0)

            def attn(sc):
                q0 = sc * SC
                ots = []
                off = sc * 4          # full (off-diagonal) k-tiles
                nkt = off + 4
                for h in range(G):
                    qrh = qr_all[:, h, q0:q0 + SC]
                    avp = ps.tile([P, SC], F32, tag="ps")
                    # two denominator accumulators: even k-tiles on Pool,
                    # odd on DVE — two independent serial chains. exp is
                    # computed as exp(s - ln16), which bounds the fp16
                    # accumulators (the 1/16 cancels between numerator and
                    # denominator); all-2-byte operands let the DVE chain
                    # run in its 2x mode
                    dnacc = [dnp.tile([P, SC], F16, tag="dn", name=f"dn{i}")
                             for i in range(2)]
                    if off == 0:
                        # first chunk: diagonal rectangles don't cover the
                        # accumulators' full width, so zero-init them
                        nc.gpsimd.memset(dnacc[0], 0.0)
                        nc.vector.memset(dnacc[1], 0.0)
                    for kt in range(nkt):
                        # diagonal k-tile kt2 only attends q columns
                        # >= 128*kt2: compute the valid rectangle only
                        c0 = (kt - off) * P if kt >= off else 0
                        ptps = pts.tile([P, SC], F32, tag="pts")
                        diag = kt >= off
                        nc.tensor.matmul(
                            ptps[:, c0:], KR[:, kt * P:(kt + 1) * P],
                            qrh[:, c0:],
                            start=True, stop=not diag,
                        )
                        if diag:
                            # additive causal mask for the leading 128 q
                            # columns (the exact-diagonal subtile)
                            nc.tensor.matmul(
                                ptps[:, c0:c0 + P], ident16, umask16,
                                start=False, stop=True,
                            )
                        pt = ptp.tile([P, SC], BF16, tag="pt")
                        nc.scalar.activation(pt[:, c0:], ptps[:, c0:],
                                             Exp, bias=bias_ln16, scale=1.0)
                        eng = nc.gpsimd if kt % 2 == 0 else nc.vector
                        acc = dnacc[kt % 2]
                        if kt < 2 and off > 0:
                            eng.tensor_copy(acc, pt)
                        else:
                            eng.tensor_add(acc[:, c0:], acc[:, c0:],
                                           pt[:, c0:])
                        nc.tensor.matmul(avp[:, c0:], Vs[:, kt, :],
                                         pt[:, c0:],
                                         start=(kt == 0),
                                         stop=(kt == nkt - 1))
                    # fold the 128 partition-partials of the denominator
                    dnps = ps.tile([P, SC], F32, tag="ps")
                    for i, acc in enumerate(dnacc):
                        nc.tensor.matmul(dnps, ones16, acc,
                                         start=(i == 0),
                                         stop=(i == len(dnacc) - 1))
                    rcp = scr.tile([P, SC], F32, tag="rcp")
                    nc.vector.reciprocal(rcp, dnps)
                    ot = otp.tile([P, SC], F16, tag="ot")
                    nc.vector.tensor_mul(ot, avp, rcp)
                    ots.append(ot)
                    if dbg:
                        nc.gpsimd.dma_start(OT_dbg[:, h, q0:q0 + SC], ot)
                return ots

            def wo_proj(sc, ots):
                q0 = sc * SC
                # output projection: 4 column groups of 2 PSUM banks
                for stl in range(SC // P):
                    srow = q0 + stl * P
                    for grp in range(4):
                        ybuf = yp.tile([P, 2 * SC], F16, tag="ys")
                        yps_l = [ps.tile([P, SC], F32, tag="ps",
                                         name=f"yps{j}")
                                 for j in range(2)]
                        for h in range(G):
                            lhs = ots[h][:, stl * P:(stl + 1) * P]
                            for j in range(2):
                                hc = grp * 2 + j
                                nc.tensor.matmul(
                                    yps_l[j], lhs,
                                    woT_sb[:, h, hc * SC:(hc + 1) * SC],
                                    start=(h == 0), stop=(h == G - 1),
                                )
                        nc.scalar.copy(ybuf[:, 0:SC], yps_l[0])
                        nc.vector.tensor_copy(ybuf[:, SC:2 * SC], yps_l[1])
                        c0 = grp * 2 * SC
                        if sc == NSC - 1 and stl == SC // P - 1 and grp == 3:
                            # last group: ship each half as its copy lands,
                            # on independent queues (Pool SWDGE + idle SP
                            # HWDGE), so the kernel-tail drain is gated by
                            # the shortest possible copy+DMA chain
                            nc.gpsimd.dma_start(
                                y[srow:srow + P, c0:c0 + SC], ybuf[:, 0:SC])
                            nc.sync.dma_start(
                                y[srow:srow + P, c0 + SC:c0 + 2 * SC],
                                ybuf[:, SC:2 * SC])
                        else:
                            nc.gpsimd.dma_start(
                                y[srow:srow + P, c0:c0 + 2 * SC], ybuf)

            # software-pipelined emission: attention of chunk sc is emitted
            # before projections of chunk sc+1, so the scheduler prefers
            # the latency-critical attention chain and fills its ACT-bound
            # gaps with projection matmuls. Chunk 0's wo is deferred to the
            # very end, where it fills the exp-bound gaps of the last
            # chunk's attention (which has no projection work left to
            # overlap with).
            proj_rope(0)
            # woT is first needed by wo(1) (~100us in); emitting its DMA
            # here keeps the 12.6us ACT-queue hold behind chunk-0's rope
            # and attention activations
            nc.scalar.dma_start(
                woT_sb, woT.rearrange("(g p) h -> p g h", p=P))
            ots0 = attn(0)
            proj_rope(1)
            ots1 = attn(1)
            proj_rope(2)
            ots2 = attn(2)
            # each wo is emitted one chunk late so its matmuls rank below
            # the next chunk's attention in scheduler priority: they stay
            # in reserve and fill the exp-rate-limited attention windows
            # instead of draining early during the projection passes
            wo_proj(1, ots1)
            proj_rope(3)
            ots3 = attn(NSC - 1)
            wo_proj(2, ots2)
            wo_proj(0, ots0)
            wo_proj(NSC - 1, ots3)

            if dbg:
                nc.sync.dma_start(KR_dbg, KR)
                nc.sync.dma_start(QR_dbg, qr_all)
                nc.sync.dma_start(V_dbg, Vs)

    nc.finalize()
    return nc


def shard_inputs(x, wq, wk, wv, wo, q_norm_w, k_norm_w, cos_table, sin_table,
                 positions, **_ignored):
    """Host-side sharding: returns the list of 8 per-core input maps."""
    x = np.asarray(x, np.float32)
    pos = np.asarray(positions).astype(np.int64)
    cos_sel = np.asarray(cos_table, np.float32)[pos]   # [S, D]
    sin_sel = np.asarray(sin_table, np.float32)[pos]
    qw = np.asarray(q_norm_w, np.float32)
    kw = np.asarray(k_norm_w, np.float32)
    # fold norm weights into the transposed rope tables:
    # w * rope(q') == q'*(w*cos) + rot(q')*(w*sin)
    # fold rotate-half's minus sign into sin rows 0..63:
    # rope(z) = z*cos + [-z2; z1]*sin = z*cos + rot(z)*sin_eff
    # and pre-rotate sin so rot(z)*sin_eff == rot(z * rot(sin_eff))
    sign = np.ones((1, D), np.float32)
    sign[0, :D // 2] = -1.0
    cosq_ = np.ascontiguousarray((cos_sel * qw).T).astype(np.float16)
    sinq_ = np.ascontiguousarray(
        np.roll((sin_sel * qw * sign).T, 64, axis=0)).astype(np.float16)
    cosk_ = np.ascontiguousarray((cos_sel * kw).T).astype(np.float16)
    sink_ = np.ascontiguousarray(
        np.roll((sin_sel * kw * sign).T, 64, axis=0)).astype(np.float16)
    xTf = np.ascontiguousarray(x.reshape(S, HID).T).astype(np.float16)
    wq = np.asarray(wq, np.float32)
    wk = np.asarray(wk, np.float32)
    wv = np.asarray(wv, np.float32)
    wo = np.asarray(wo, np.float32)

    in_maps = []
    for c in range(N_CORES):
        wq_c = wq[c * G * P:(c + 1) * G * P, :].T        # [HID, 512]
        wk_c = wk[c * P:(c + 1) * P, :].T                # [HID, 128]
        wv_c = wv[c * P:(c + 1) * P, :].T                # [HID, 128]
        wqkv = np.concatenate([wq_c, wk_c, wv_c], axis=1)
        m = {
            "xT": xTf,
            "wqkvT": np.ascontiguousarray(wqkv).astype(np.float16),
            "woT": np.ascontiguousarray(
                wo[:, c * G * P:(c + 1) * G * P].T).astype(np.float16),
            "cosq": cosq_, "sinq": sinq_, "cosk": cosk_, "sink": sink_,
        }
        in_maps.append(m)
    return in_maps


_NC = None


def _get_nc():
    global _NC
    if _NC is None:
        _NC = build_program()
    return _NC


def run_on_device(in_maps, trace=False):
    from concourse.bass_utils import run_bass_kernel_spmd
    nc = _get_nc()
    return run_bass_kernel_spmd(nc, in_maps, list(range(N_CORES)), trace=trace)


def kernel(**inputs):
    in_maps = shard_inputs(**inputs)
    res = run_on_device(in_maps).results
    y = np.zeros((S, HID), np.float32)
    for c in range(N_CORES):
        y += res[c]["y"].astype(np.float32)
    return y.reshape(1, S, HID)
